# revision 8
# baseline (speedup 1.0000x reference)
"""Trainium2 Bass kernel for CdfgReader GNN message passing.

Strategy:
  - Node features depend only on which CDFG a batch item references. With 64
    batch items over 32 CDFGs, run the GNN once per UNIQUE graph (<=32),
    4 graph slots per core across 8 cores. Each core emits the [64, 256]
    rows for batch items whose graph it owns; the host gathers rows.
  - Per slot: X0 = relu(xs @ W_in + b), 4 GCN layers (A @ (X @ W) + b,
    relu/tanh), residual, masked mean via a small mask matmul whose mask
    columns are pre-scaled by 1/count on the host.
  - Precision (rel tol 2e-2, ~5e-3 predicted): X@W matmuls run in f32r as
    X_hi @ (W_hi + W_lo) with W split at 11 mantissa bits. The dominant
    A-multiplies run in fp8e4 with perf_mode=DoubleRow (256-wide
    contraction at double rate): A is 0/1 (exact in fp8), and XW is split
    into two e4m3 parts (~8 effective mantissa bits) after scaling by a
    per-layer power of two (folded into W on the host, un-scaled in the
    activation) so values sit in e4m3's dynamic range.
  - Engine balance: PE matmuls; ACT does the fp8 hi-conversions + scaled
    relu/tanh; DVE does the fp8 lo-parts + input-layer relus; the
    otherwise-idle Pool engine does the residual adds.
  - Emission is software-pipelined across graphs so the PE never waits for
    activation/convert tails at layer or graph boundaries.
"""

import numpy as np

NG = 4          # graph slots per core
NCORES = 8
N = 1024        # max nodes
F = 128         # input feature dim
H = 256         # hidden dim
L = 4           # GCN layers
B = 64          # batch (coverpoints)

_CACHE = {}


def _build_nc(with_bias, inv_scales):
    import concourse.bass as bass  # noqa: F401
    import concourse.mybir as mybir
    import concourse.tile as tile
    from concourse import bacc
    from concourse.bass import ts

    f32 = mybir.dt.float32
    f32r = mybir.dt.float32r
    f8 = mybir.dt.float8e4
    DR = mybir.MatmulPerfMode.DoubleRow
    Relu = mybir.ActivationFunctionType.Relu
    Tanh = mybir.ActivationFunctionType.Tanh
    Copy = mybir.ActivationFunctionType.Copy
    sub = mybir.AluOpType.subtract

    nc = bacc.Bacc("TRN2", target_bir_lowering=False, debug=False,
                   num_devices=NCORES)

    a_t = nc.dram_tensor("a_t", [NG, 128, 4, 2 * N], f8, kind="ExternalInput")
    xs_t_hi = nc.dram_tensor("xs_t_hi", [F, NG, N], f32r, kind="ExternalInput")
    m_t = nc.dram_tensor("m_t", [128, NG * 8, B], f32r, kind="ExternalInput")
    w_in_hl = nc.dram_tensor("w_in_hl", [F, 2 * H + 2 + L * 2], f32r,
                             kind="ExternalInput")
    w_gcn_hi = nc.dram_tensor("w_gcn_hi", [128, L * 2, H], f32r,
                              kind="ExternalInput")
    w_gcn_lo = nc.dram_tensor("w_gcn_lo", [128, L * 2, H], f32r,
                              kind="ExternalInput")
    b_in_row = nc.dram_tensor("b_in_row", [1, H], f32r, kind="ExternalInput")
    b_g3_row = nc.dram_tensor("b_g3_row", [1, H], f32r, kind="ExternalInput")
    ones_row = nc.dram_tensor("ones_row", [1, 128], f32r, kind="ExternalInput")
    out = nc.dram_tensor("out", [B, H], f32, kind="ExternalOutput")

    with tile.TileContext(nc) as tc:
        with (
            tc.tile_pool(name="const", bufs=1) as constp,
            tc.tile_pool(name="apool", bufs=2) as apool,
            tc.tile_pool(name="xpool", bufs=2) as xpool,
            tc.tile_pool(name="xhpool", bufs=3) as xhpool,
            tc.tile_pool(name="xtpool", bufs=3) as xtpool,
            tc.tile_pool(name="x8pool", bufs=2) as x8pool,
            tc.tile_pool(name="psx", bufs=4, space="PSUM") as psx,
            tc.tile_pool(name="psw", bufs=3, space="PSUM") as psw,
            tc.tile_pool(name="psm", bufs=1, space="PSUM") as psm,
        ):
            # --- head DMAs in consumption order ---
            # head: [w_in_hi (256) | w_in_lo (256) | b_pp (10)] in one DMA
            head_sb = constp.tile([128, 2 * H + 2 + L * 2], f32r)
            nc.sync.dma_start(head_sb[:], w_in_hl[:, :])
            if with_bias:
                b_in_row_sb = constp.tile([1, H], f32r)
                nc.sync.dma_start(b_in_row_sb[:], b_in_row[:, :])
                b_g3_row_sb = constp.tile([1, H], f32r)
                nc.sync.dma_start(b_g3_row_sb[:], b_g3_row[:, :])
                ones_sb = constp.tile([1, 128], f32r)
                nc.sync.dma_start(ones_sb[:], ones_row[:, :])

            w_hi_sb = constp.tile([128, L * 2, H], f32r)
            w_lo_sb = constp.tile([128, L * 2, H], f32r)
            m_t_sb = constp.tile([128, NG * 8, B], f32r)
            out_acc = constp.tile([B, H], f32)

            xs_tiles = [None] * NG
            a_tiles = [None] * NG

            def emit_dma(g):
                xs_g = xpool.tile([128, N], f32r, tag="xs", name=f"xs{g}")
                for c in range(2):
                    nc.sync.dma_start(xs_g[:, ts(c, 512)],
                                      xs_t_hi[:, g, ts(c, 512)])
                xs_tiles[g] = xs_g
                a8 = apool.tile([128, 4, 2, N], f8, tag="a", name=f"a{g}")
                if g == 0:
                    nc.sync.dma_start(w_hi_sb[:, 0:2, :], w_gcn_hi[:, 0:2, :])
                    nc.sync.dma_start(w_lo_sb[:, 0:2, :], w_gcn_lo[:, 0:2, :])
                    for q in range(4):
                        nc.sync.dma_start(
                            a8[:, q, :, :].rearrange("p ko i -> p (ko i)"),
                            a_t[0, :, q, :])
                    for lyr in range(1, L):
                        nc.sync.dma_start(w_hi_sb[:, 2 * lyr:2 * lyr + 2, :],
                                          w_gcn_hi[:, 2 * lyr:2 * lyr + 2, :])
                        nc.sync.dma_start(w_lo_sb[:, 2 * lyr:2 * lyr + 2, :],
                                          w_gcn_lo[:, 2 * lyr:2 * lyr + 2, :])
                    nc.sync.dma_start(m_t_sb[:], m_t[:, :, :])
                else:
                    nc.sync.dma_start(
                        a8[:].rearrange("p q ko i -> p q (ko i)"), a_t[g])
                a_tiles[g] = a8

            x_cur = [None] * NG
            x0n_tiles = [None] * NG
            xw_tiles = [None] * NG
            xf_tiles = [None] * NG

            def emit_p(g):
                """x0t (h-major input layer) + x0n (node-major residual)."""
                xs_g = xs_tiles[g]
                x0t = xhpool.tile([128, 2, N], f32r, tag="xh", name=f"x0t{g}")
                for c in range(2):
                    for t in range(2):
                        ps = psx.tile([128, 512], mybir.dt.float32, tag="psx")
                        nc.tensor.matmul(ps[:], head_sb[:, ts(t, 128)],
                                         xs_g[:, ts(c, 512)],
                                         start=True, stop=False)
                        nc.tensor.matmul(ps[:],
                                         head_sb[:, 256 + t * 128:
                                                 256 + (t + 1) * 128],
                                         xs_g[:, ts(c, 512)],
                                         start=False, stop=True)
                        nc.scalar.activation(
                            x0t[:, t, ts(c, 512)], ps[:], Relu,
                            bias=head_sb[:, 512 + t:513 + t])
                x0n = xpool.tile([128, 8, H], f32, tag="x0n", name=f"x0n{g}")
                for i in range(8):
                    ps = psw.tile([128, H], mybir.dt.float32, tag="psw")
                    nc.tensor.matmul(ps[:], xs_g[:, ts(i, 128)],
                     head_sb[:, 0:256],
                                     start=True, stop=not with_bias)
                    if with_bias:
                        nc.tensor.matmul(ps[:], ones_sb[:], b_in_row_sb[:],
                                         start=False, stop=True)
                    nc.scalar.activation(x0n[:, i, :], ps[:], Relu)
                x_cur[g] = x0t
                x0n_tiles[g] = x0n

            def emit_w(g, layer):
                """s_l*XW = X_hi @ (W_hi + W_lo) -> split to two fp8 parts."""
                x = x_cur[g]
                xw_hi = x8pool.tile([128, 8, H], f8, tag="x8h",
                                    name=f"xw8h{g}_{layer}")
                xw_lo = x8pool.tile([128, 8, H], f8, tag="x8l",
                                    name=f"xw8l{g}_{layer}")
                for m in range(8):
                    ps = psw.tile([128, H], mybir.dt.float32, tag="psw")
                    k = 0
                    for t in range(2):
                        for w_sb in (w_hi_sb, w_lo_sb):
                            nc.tensor.matmul(
                                ps[:], x[:, t, ts(m, 128)],
                                w_sb[:, layer * 2 + t, :],
                                start=(k == 0), stop=(k == 3))
                            k += 1
                    if m % 3 == 0:
                        nc.scalar.activation(xw_hi[:, m, :], ps[:], Copy)
                    else:
                        nc.vector.tensor_copy(xw_hi[:, m, :], ps[:])
                    nc.vector.tensor_tensor(xw_lo[:, m, :], ps[:],
                                            xw_hi[:, m, :], sub)
                xw_tiles[g] = (xw_hi, xw_lo)

            def emit_a(g, layer):
                (xw_hi, xw_lo), a8 = xw_tiles[g], a_tiles[g]
                inv_s = float(inv_scales[layer])
                if layer < L - 1:
                    # X_next^T[h, i] = sum_m XW[m, h] * A^T[m, i] (DoubleRow)
                    xn = xhpool.tile([128, 2, N], f32r, tag="xh",
                                     name=f"xn{g}_{layer}")
                    for c in range(2):
                        for t in range(2):
                            ps = psx.tile([128, 512], mybir.dt.float32,
                                          tag="psx")
                            for k, part in enumerate((xw_hi, xw_lo)):
                                for q in range(4):
                                    nc.tensor.matmul(
                                        ps[:],
                                        part[:, 2 * q:2 * q + 2, ts(t, 128)],
                                        a8[:, q, :, ts(c, 512)],
                                        start=(k == 0 and q == 0),
                                        stop=(k == 1 and q == 3),
                                        perf_mode=DR)
                            nc.scalar.activation(
                                xn[:, t, ts(c, 512)], ps[:], Relu,
                                bias=head_sb[:, 514 + layer * 2 + t:
                                             515 + layer * 2 + t],
                                scale=inv_s)
                    x_cur[g] = xn
                else:
                    # final layer node-major: tanh to SBUF, residual on Pool
                    xf = xpool.tile([128, 8, H], f32r, tag="xf",
                                    name=f"xf{g}")
                    x0n = x0n_tiles[g]
                    for i in range(8):
                        ps = psw.tile([128, H], mybir.dt.float32, tag="psw")
                        for k, part in enumerate((xw_hi, xw_lo)):
                            for q in range(4):
                                nc.tensor.matmul(
                                    ps[:], a8[:, q, :, ts(i, 128)],
                                    part[:, 2 * q:2 * q + 2, :],
                                    start=(k == 0 and q == 0),
                                    stop=(k == 1 and q == 3
                                          and not with_bias),
                                    perf_mode=DR)
                        if with_bias:
                            nc.tensor.matmul(ps[:], ones_sb[:],
                                             b_g3_row_sb[:],
                                             start=False, stop=True)
                        xt = xtpool.tile([128, H], f32, tag="xt3")
                        nc.scalar.activation(xt[:], ps[:], Tanh, scale=inv_s)
                        nc.gpsimd.tensor_add(xf[:, i, :], xt[:],
                                             x0n[:, i, :])
                    xf_tiles[g] = xf

            def emit_m(g):
                """masked (pre-scaled) sums: psum[b, h] += M^T @ Xf."""
                xf = xf_tiles[g]
                pm = psm.tile([B, H], mybir.dt.float32, tag="psm")
                for c in range(8):
                    nc.tensor.matmul(pm[:], m_t_sb[:, g * 8 + c, :],
                                     xf[:, c, :], start=(c == 0), stop=(c == 7))
                if g == 0:
                    nc.vector.tensor_copy(out_acc[:], pm[:])
                else:
                    nc.vector.tensor_add(out_acc[:], out_acc[:], pm[:])

            # --- software-pipelined emission ---
            emit_dma(0)
            emit_p(0)
            emit_w(0, 0)
            for g in range(NG):
                emit_a(g, 0)
                emit_w(g, 1)
                if g + 1 < NG:
                    emit_dma(g + 1)
                emit_a(g, 1)
                emit_w(g, 2)
                emit_a(g, 2)
                emit_w(g, 3)
                if g + 1 < NG:
                    emit_p(g + 1)
                emit_a(g, 3)
                if g + 1 < NG:
                    emit_w(g + 1, 0)
                emit_m(g)

            nc.sync.dma_start(out[:, :], out_acc[:])

    nc.compile()
    return nc


def _get_nc(with_bias, inv_scales):
    key = ("nc", bool(with_bias), tuple(inv_scales))
    if key not in _CACHE:
        _CACHE[key] = _build_nc(with_bias, inv_scales)
    return _CACHE[key]


def _rnd11(x):
    # round-to-nearest-even at 11 explicit mantissa bits (f32r-exact)
    m, e = np.frexp(np.float32(x))
    m = np.round(m * 4096.0) / 4096.0
    return np.ldexp(m, e).astype(np.float32)


def _layer_scales(cdfg_xs, cdfg_as, uniq, W_in, b_in, W_gcn, b_gcn):
    """Power-of-two per-layer scales s_l with max|s_l * XW_l| <= ~110
    (e4m3 max is 240), from an fp32 forward pass over the unique graphs."""
    maxs = np.zeros(L, dtype=np.float64)
    for g in uniq:
        x = np.maximum(cdfg_xs[g] @ W_in + b_in, 0.0).astype(np.float32)
        a = cdfg_as[g]
        for l in range(L):
            xw = x @ W_gcn[l]
            maxs[l] = max(maxs[l], float(np.abs(xw).max()))
            h = a @ xw + b_gcn[l]
            x = (np.maximum(h, 0.0) if l < L - 1
                 else np.tanh(h)).astype(np.float32)
    s = np.exp2(np.clip(np.floor(np.log2(110.0 / np.maximum(maxs, 1e-30))),
                        -30, 30))
    return s.astype(np.float64)


def _prepare_in_maps(cdfg_xs, cdfg_as, graph, coverpoint_mask,
                     W_in, b_in, W_gcn, b_gcn):
    import concourse.mybir as mybir
    f8np = mybir.dt.np(mybir.dt.float8e4)

    cdfg_xs = np.asarray(cdfg_xs, dtype=np.float32)
    cdfg_as = np.asarray(cdfg_as, dtype=np.float32)
    graph = np.asarray(graph).astype(np.int64)
    maskf = np.asarray(coverpoint_mask).astype(np.float32)
    W_in = np.asarray(W_in, dtype=np.float32)
    b_in = np.asarray(b_in, dtype=np.float32)
    W_gcn = np.asarray(W_gcn, dtype=np.float32)
    b_gcn = np.asarray(b_gcn, dtype=np.float32)
    with_bias = bool(np.any(b_in) or np.any(b_gcn))

    uniq = np.unique(graph)
    nslots = NG * NCORES
    slots = np.empty(nslots, dtype=np.int64)
    slots[:len(uniq)] = uniq
    slots[len(uniq):] = uniq[0]
    real = np.zeros(nslots, dtype=bool)
    real[:len(uniq)] = True

    scales = _layer_scales(cdfg_xs, cdfg_as, uniq, W_in, b_in, W_gcn, b_gcn)
    inv_scales = (1.0 / scales).astype(np.float64)

    w_gcn_layout = np.ascontiguousarray(
        W_gcn.reshape(L, 2, 128, H).transpose(2, 0, 1, 3)
        .reshape(128, L * 2, H))
    w_gcn_hi = _rnd11(w_gcn_layout)
    w_gcn_lo = _rnd11(w_gcn_layout - w_gcn_hi)
    # fold the per-layer fp8 scale into the (power-of-two-exact) weights
    sc = np.repeat(scales.astype(np.float32), 2)  # [L*2]
    w_gcn_hi = np.ascontiguousarray(w_gcn_hi * sc[None, :, None])
    w_gcn_lo = np.ascontiguousarray(w_gcn_lo * sc[None, :, None])
    w_in_hi = _rnd11(W_in)
    w_in_lo = _rnd11(W_in - w_in_hi)
    b_pp = np.concatenate(
        [b_in.reshape(2, 128).T,
         b_gcn.reshape(L, 2, 128).transpose(2, 0, 1).reshape(128, L * 2)],
        axis=1)
    w_in_hl = np.ascontiguousarray(
        np.concatenate([w_in_hi, w_in_lo, b_pp], axis=1))
    mscale = maskf / maskf.sum(axis=1, keepdims=True)

    common = {
        "w_in_hl": w_in_hl,
        "w_gcn_hi": w_gcn_hi,
        "w_gcn_lo": w_gcn_lo,
        "b_in_row": np.ascontiguousarray(b_in.reshape(1, H)),
        "b_g3_row": np.ascontiguousarray(
            (b_gcn[L - 1] * scales[L - 1]).reshape(1, H)),
        "ones_row": np.ones((1, 128), dtype=np.float32),
    }

    in_maps = []
    for k in range(NCORES):
        sl = slots[k * NG:(k + 1) * NG]
        a_t = np.empty((NG, 128, 4, 2 * N), dtype=f8np)
        for g in range(NG):
            # [m, i] -> [q, ko, p, i] -> [p, q, (ko i)]
            at = cdfg_as[sl[g]].T.reshape(4, 2, 128, N)
            a_t[g] = at.transpose(2, 0, 1, 3).reshape(
                128, 4, 2 * N).astype(f8np)
        xs_t_hi = _rnd11(np.ascontiguousarray(cdfg_xs[sl].transpose(2, 0, 1)))
        m_t = np.zeros((128, NG * 8, B), dtype=np.float32)
        for g in range(NG):
            if real[k * NG + g]:
                rows = np.nonzero(graph == sl[g])[0]
                for b in rows:
                    m_t[:, g * 8:(g + 1) * 8, b] = mscale[b].reshape(8, 128).T
        in_maps.append({"a_t": a_t, "xs_t_hi": xs_t_hi,
                        "m_t": m_t, **common})
    return in_maps, slots, real, with_bias, inv_scales


def _assemble_out(results, graph, slots, real):
    graph = np.asarray(graph).astype(np.int64)
    out = np.zeros((B, H), dtype=np.float32)
    for k in range(NCORES):
        for g in range(NG):
            if real[k * NG + g]:
                rows = graph == slots[k * NG + g]
                out[rows] = results[k]["out"][rows]
    return out


def kernel(cdfg_xs, cdfg_as, graph, coverpoint_mask, W_in, b_in, W_gcn, b_gcn):
    from concourse.bass_utils import run_bass_kernel_spmd

    in_maps, slots, real, with_bias, inv_scales = _prepare_in_maps(
        cdfg_xs, cdfg_as, graph, coverpoint_mask, W_in, b_in, W_gcn, b_gcn)
    nc = _get_nc(with_bias, inv_scales)
    res = run_bass_kernel_spmd(nc, in_maps, core_ids=list(range(NCORES)))
    return _assemble_out(res.results, graph, slots, real)


# revision 9
# speedup vs baseline: 1.1420x; 1.1420x over previous
"""Trainium2 Bass kernel for CdfgReader GNN message passing.

Strategy:
  - Node features depend only on which CDFG a batch item references. With 64
    batch items over 32 CDFGs, run the GNN once per UNIQUE graph (<=32),
    4 graph slots per core across 8 cores. Each core emits the [64, 256]
    rows for batch items whose graph it owns; the host gathers rows.
  - Per slot: X0 = relu(xs @ W_in + b), 4 GCN layers (A @ (X @ W) + b,
    relu/tanh), residual, masked mean via a small mask matmul whose mask
    columns are pre-scaled by 1/count on the host.
  - Precision (rel tol 2e-2, ~5e-3 predicted): X@W matmuls run in f32r as
    X_hi @ (W_hi + W_lo) with W split at 11 mantissa bits. The dominant
    A-multiplies run in fp8e4 with perf_mode=DoubleRow (256-wide
    contraction at double rate): A is 0/1 (exact in fp8), and XW is split
    into two e4m3 parts (~8 effective mantissa bits) after scaling by a
    per-layer power of two (folded into W on the host, un-scaled in the
    activation) so values sit in e4m3's dynamic range.
  - Engine balance: PE matmuls; ACT does the fp8 hi-conversions + scaled
    relu/tanh; DVE does the fp8 lo-parts + input-layer relus; the
    otherwise-idle Pool engine does the residual adds.
  - Emission is software-pipelined across graphs so the PE never waits for
    activation/convert tails at layer or graph boundaries.
"""

import numpy as np

NG = 4          # graph slots per core
NCORES = 8
N = 1024        # max nodes
F = 128         # input feature dim
H = 256         # hidden dim
L = 4           # GCN layers
B = 64          # batch (coverpoints)

_CACHE = {}


def _build_nc(with_bias, inv_scales):
    import concourse.bass as bass  # noqa: F401
    import concourse.mybir as mybir
    import concourse.tile as tile
    from concourse import bacc
    from concourse.bass import ts

    f32 = mybir.dt.float32
    f32r = mybir.dt.float32r
    f8 = mybir.dt.float8e4
    DR = mybir.MatmulPerfMode.DoubleRow
    Relu = mybir.ActivationFunctionType.Relu
    Tanh = mybir.ActivationFunctionType.Tanh
    Copy = mybir.ActivationFunctionType.Copy
    sub = mybir.AluOpType.subtract

    nc = bacc.Bacc("TRN2", target_bir_lowering=False, debug=False,
                   num_devices=NCORES)

    a_t = nc.dram_tensor("a_t", [NG, 128, 4, 2 * N], f8, kind="ExternalInput")
    xs_t_hi = nc.dram_tensor("xs_t_hi", [F, NG, N], f32r, kind="ExternalInput")
    m_t = nc.dram_tensor("m_t", [128, NG * 8, B], f32r, kind="ExternalInput")
    w_in_hl = nc.dram_tensor("w_in_hl", [F, 2 * H + 2 + L * 2], f32r,
                             kind="ExternalInput")
    w_gcn_hi = nc.dram_tensor("w_gcn_hi", [128, L * 2, H], f32r,
                              kind="ExternalInput")
    w_gcn_lo = nc.dram_tensor("w_gcn_lo", [128, L * 2, H], f32r,
                              kind="ExternalInput")
    b_in_row = nc.dram_tensor("b_in_row", [1, H], f32r, kind="ExternalInput")
    b_g3_row = nc.dram_tensor("b_g3_row", [1, H], f32r, kind="ExternalInput")
    ones_row = nc.dram_tensor("ones_row", [1, 128], f32r, kind="ExternalInput")
    out = nc.dram_tensor("out", [B, H], f32, kind="ExternalOutput")

    with tile.TileContext(nc) as tc:
        with (
            tc.tile_pool(name="const", bufs=1) as constp,
            tc.tile_pool(name="apool", bufs=2) as apool,
            tc.tile_pool(name="xpool", bufs=2) as xpool,
            tc.tile_pool(name="xhpool", bufs=3) as xhpool,
            tc.tile_pool(name="xtpool", bufs=3) as xtpool,
            tc.tile_pool(name="x8pool", bufs=2) as x8pool,
            tc.tile_pool(name="psx", bufs=3, space="PSUM") as psx,
            tc.tile_pool(name="psw", bufs=4, space="PSUM") as psw,
            tc.tile_pool(name="psm", bufs=1, space="PSUM") as psm,
        ):
            # --- head DMAs in consumption order ---
            # head: [w_in_hi (256) | w_in_lo (256) | b_pp (10)] in one DMA
            head_sb = constp.tile([128, 2 * H + 2 + L * 2], f32r)
            nc.sync.dma_start(head_sb[:], w_in_hl[:, :])
            if with_bias:
                b_in_row_sb = constp.tile([1, H], f32r)
                nc.sync.dma_start(b_in_row_sb[:], b_in_row[:, :])
                b_g3_row_sb = constp.tile([1, H], f32r)
                nc.sync.dma_start(b_g3_row_sb[:], b_g3_row[:, :])
                ones_sb = constp.tile([1, 128], f32r)
                nc.sync.dma_start(ones_sb[:], ones_row[:, :])

            w_hi_sb = constp.tile([128, L * 2, H], f32r)
            w_lo_sb = constp.tile([128, L * 2, H], f32r)
            m_t_sb = constp.tile([128, NG * 8, B], f32r)
            out_acc = constp.tile([B, H], f32)

            xs_tiles = [None] * NG
            a_tiles = [None] * NG

            def emit_dma(g):
                xs_g = xpool.tile([128, N], f32r, tag="xs", name=f"xs{g}")
                for c in range(2):
                    nc.sync.dma_start(xs_g[:, ts(c, 512)],
                                      xs_t_hi[:, g, ts(c, 512)])
                xs_tiles[g] = xs_g
                a8 = apool.tile([128, 4, 2, N], f8, tag="a", name=f"a{g}")
                if g == 0:
                    nc.sync.dma_start(w_hi_sb[:, 0:2, :], w_gcn_hi[:, 0:2, :])
                    nc.sync.dma_start(w_lo_sb[:, 0:2, :], w_gcn_lo[:, 0:2, :])
                    for q in range(4):
                        nc.sync.dma_start(
                            a8[:, q, :, :].rearrange("p ko i -> p (ko i)"),
                            a_t[0, :, q, :])
                    for lyr in range(1, L):
                        nc.sync.dma_start(w_hi_sb[:, 2 * lyr:2 * lyr + 2, :],
                                          w_gcn_hi[:, 2 * lyr:2 * lyr + 2, :])
                        nc.sync.dma_start(w_lo_sb[:, 2 * lyr:2 * lyr + 2, :],
                                          w_gcn_lo[:, 2 * lyr:2 * lyr + 2, :])
                    nc.sync.dma_start(m_t_sb[:], m_t[:, :, :])
                else:
                    nc.sync.dma_start(
                        a8[:].rearrange("p q ko i -> p q (ko i)"), a_t[g])
                a_tiles[g] = a8

            x_cur = [None] * NG
            x0n_tiles = [None] * NG
            xw_tiles = [None] * NG
            xf_tiles = [None] * NG

            def emit_p_x0t(g):
                xs_g = xs_tiles[g]
                x0t = xhpool.tile([128, 2, N], f32r, tag="xh", name=f"x0t{g}")
                for c in range(2):
                    for t in range(2):
                        ps = psx.tile([128, 512], mybir.dt.float32, tag="psx")
                        nc.tensor.matmul(ps[:], head_sb[:, ts(t, 128)],
                                         xs_g[:, ts(c, 512)],
                                         start=True, stop=False)
                        nc.tensor.matmul(ps[:],
                                         head_sb[:, 256 + t * 128:
                                                 256 + (t + 1) * 128],
                                         xs_g[:, ts(c, 512)],
                                         start=False, stop=True)
                        nc.scalar.activation(
                            x0t[:, t, ts(c, 512)], ps[:], Relu,
                            bias=head_sb[:, 512 + t:513 + t])
                x_cur[g] = x0t

            def emit_p_x0n(g):
                xs_g = xs_tiles[g]
                x0n = xpool.tile([128, 8, H], f32, tag="x0n", name=f"x0n{g}")
                for i in range(8):
                    ps = psw.tile([128, H], mybir.dt.float32, tag="psw")
                    nc.tensor.matmul(ps[:], xs_g[:, ts(i, 128)],
                     head_sb[:, 0:256],
                                     start=True, stop=not with_bias)
                    if with_bias:
                        nc.tensor.matmul(ps[:], ones_sb[:], b_in_row_sb[:],
                                         start=False, stop=True)
                    nc.scalar.activation(x0n[:, i, :], ps[:], Relu)
                x0n_tiles[g] = x0n

            def emit_w(g, layer):
                """s_l*XW = X_hi @ (W_hi + W_lo) -> split to two fp8 parts."""
                x = x_cur[g]
                xw_hi = x8pool.tile([128, 8, H], f8, tag="x8h",
                                    name=f"xw8h{g}_{layer}")
                xw_lo = x8pool.tile([128, 8, H], f8, tag="x8l",
                                    name=f"xw8l{g}_{layer}")
                for m in range(8):
                    ps = psw.tile([128, H], mybir.dt.float32, tag="psw")
                    k = 0
                    for t in range(2):
                        for w_sb in (w_hi_sb, w_lo_sb):
                            nc.tensor.matmul(
                                ps[:], x[:, t, ts(m, 128)],
                                w_sb[:, layer * 2 + t, :],
                                start=(k == 0), stop=(k == 3))
                            k += 1
                    nc.scalar.activation(xw_hi[:, m, :], ps[:], Copy)
                    nc.vector.tensor_tensor(xw_lo[:, m, :], ps[:],
                                            xw_hi[:, m, :], sub)
                xw_tiles[g] = (xw_hi, xw_lo)

            def emit_a(g, layer):
                (xw_hi, xw_lo), a8 = xw_tiles[g], a_tiles[g]
                inv_s = float(inv_scales[layer])
                if layer < L - 1:
                    # X_next^T[h, i] = sum_m XW[m, h] * A^T[m, i] (DoubleRow)
                    xn = xhpool.tile([128, 2, N], f32r, tag="xh",
                                     name=f"xn{g}_{layer}")
                    for c in range(2):
                        for t in range(2):
                            ps = psx.tile([128, 512], mybir.dt.float32,
                                          tag="psx")
                            for k, part in enumerate((xw_hi, xw_lo)):
                                for q in range(4):
                                    nc.tensor.matmul(
                                        ps[:],
                                        part[:, 2 * q:2 * q + 2, ts(t, 128)],
                                        a8[:, q, :, ts(c, 512)],
                                        start=(k == 0 and q == 0),
                                        stop=(k == 1 and q == 3),
                                        perf_mode=DR)
                            nc.scalar.activation(
                                xn[:, t, ts(c, 512)], ps[:], Relu,
                                bias=head_sb[:, 514 + layer * 2 + t:
                                             515 + layer * 2 + t],
                                scale=inv_s)
                    x_cur[g] = xn
                else:
                    # final layer node-major: tanh to SBUF, residual on Pool
                    xf = xpool.tile([128, 8, H], f32r, tag="xf",
                                    name=f"xf{g}")
                    x0n = x0n_tiles[g]
                    for i in range(8):
                        ps = psw.tile([128, H], mybir.dt.float32, tag="psw")
                        for k, part in enumerate((xw_hi, xw_lo)):
                            for q in range(4):
                                nc.tensor.matmul(
                                    ps[:], a8[:, q, :, ts(i, 128)],
                                    part[:, 2 * q:2 * q + 2, :],
                                    start=(k == 0 and q == 0),
                                    stop=(k == 1 and q == 3
                                          and not with_bias),
                                    perf_mode=DR)
                        if with_bias:
                            nc.tensor.matmul(ps[:], ones_sb[:],
                                             b_g3_row_sb[:],
                                             start=False, stop=True)
                        xt = xtpool.tile([128, H], f32, tag="xt3")
                        nc.scalar.activation(xt[:], ps[:], Tanh, scale=inv_s)
                        nc.vector.tensor_add(xf[:, i, :], xt[:],
                                             x0n[:, i, :])
                    xf_tiles[g] = xf

            def emit_m(g):
                """masked (pre-scaled) sums: psum[b, h] += M^T @ Xf."""
                xf = xf_tiles[g]
                pm = psm.tile([B, H], mybir.dt.float32, tag="psm")
                for c in range(8):
                    nc.tensor.matmul(pm[:], m_t_sb[:, g * 8 + c, :],
                                     xf[:, c, :], start=(c == 0), stop=(c == 7))
                if g == 0:
                    nc.vector.tensor_copy(out_acc[:], pm[:])
                else:
                    nc.vector.tensor_add(out_acc[:], out_acc[:], pm[:])

            # --- software-pipelined emission ---
            emit_dma(0)
            emit_p_x0t(0)
            emit_p_x0n(0)
            emit_w(0, 0)
            for g in range(NG):
                emit_a(g, 0)
                emit_w(g, 1)
                if g + 1 < NG:
                    emit_dma(g + 1)
                emit_a(g, 1)
                emit_w(g, 2)
                emit_a(g, 2)
                if g + 1 < NG:
                    emit_p_x0t(g + 1)
                emit_w(g, 3)
                emit_a(g, 3)
                if g + 1 < NG:
                    emit_w(g + 1, 0)
                emit_m(g)
                if g + 1 < NG:
                    emit_p_x0n(g + 1)

            nc.sync.dma_start(out[:, :], out_acc[:])

    nc.compile()
    return nc


def _get_nc(with_bias, inv_scales):
    key = ("nc", bool(with_bias), tuple(inv_scales))
    if key not in _CACHE:
        _CACHE[key] = _build_nc(with_bias, inv_scales)
    return _CACHE[key]


def _rnd11(x):
    # round-to-nearest-even at 11 explicit mantissa bits (f32r-exact)
    m, e = np.frexp(np.float32(x))
    m = np.round(m * 4096.0) / 4096.0
    return np.ldexp(m, e).astype(np.float32)


def _layer_scales(cdfg_xs, cdfg_as, uniq, W_in, b_in, W_gcn, b_gcn):
    """Power-of-two per-layer scales s_l with max|s_l * XW_l| <= ~110
    (e4m3 max is 240), from an fp32 forward pass over the unique graphs."""
    maxs = np.zeros(L, dtype=np.float64)
    for g in uniq:
        x = np.maximum(cdfg_xs[g] @ W_in + b_in, 0.0).astype(np.float32)
        a = cdfg_as[g]
        for l in range(L):
            xw = x @ W_gcn[l]
            maxs[l] = max(maxs[l], float(np.abs(xw).max()))
            h = a @ xw + b_gcn[l]
            x = (np.maximum(h, 0.0) if l < L - 1
                 else np.tanh(h)).astype(np.float32)
    s = np.exp2(np.clip(np.floor(np.log2(110.0 / np.maximum(maxs, 1e-30))),
                        -30, 30))
    return s.astype(np.float64)


def _prepare_in_maps(cdfg_xs, cdfg_as, graph, coverpoint_mask,
                     W_in, b_in, W_gcn, b_gcn):
    import concourse.mybir as mybir
    f8np = mybir.dt.np(mybir.dt.float8e4)

    cdfg_xs = np.asarray(cdfg_xs, dtype=np.float32)
    cdfg_as = np.asarray(cdfg_as, dtype=np.float32)
    graph = np.asarray(graph).astype(np.int64)
    maskf = np.asarray(coverpoint_mask).astype(np.float32)
    W_in = np.asarray(W_in, dtype=np.float32)
    b_in = np.asarray(b_in, dtype=np.float32)
    W_gcn = np.asarray(W_gcn, dtype=np.float32)
    b_gcn = np.asarray(b_gcn, dtype=np.float32)
    with_bias = bool(np.any(b_in) or np.any(b_gcn))

    uniq = np.unique(graph)
    nslots = NG * NCORES
    slots = np.empty(nslots, dtype=np.int64)
    slots[:len(uniq)] = uniq
    slots[len(uniq):] = uniq[0]
    real = np.zeros(nslots, dtype=bool)
    real[:len(uniq)] = True

    scales = _layer_scales(cdfg_xs, cdfg_as, uniq, W_in, b_in, W_gcn, b_gcn)
    inv_scales = (1.0 / scales).astype(np.float64)

    w_gcn_layout = np.ascontiguousarray(
        W_gcn.reshape(L, 2, 128, H).transpose(2, 0, 1, 3)
        .reshape(128, L * 2, H))
    w_gcn_hi = _rnd11(w_gcn_layout)
    w_gcn_lo = _rnd11(w_gcn_layout - w_gcn_hi)
    # fold the per-layer fp8 scale into the (power-of-two-exact) weights
    sc = np.repeat(scales.astype(np.float32), 2)  # [L*2]
    w_gcn_hi = np.ascontiguousarray(w_gcn_hi * sc[None, :, None])
    w_gcn_lo = np.ascontiguousarray(w_gcn_lo * sc[None, :, None])
    w_in_hi = _rnd11(W_in)
    w_in_lo = _rnd11(W_in - w_in_hi)
    b_pp = np.concatenate(
        [b_in.reshape(2, 128).T,
         b_gcn.reshape(L, 2, 128).transpose(2, 0, 1).reshape(128, L * 2)],
        axis=1)
    w_in_hl = np.ascontiguousarray(
        np.concatenate([w_in_hi, w_in_lo, b_pp], axis=1))
    mscale = maskf / maskf.sum(axis=1, keepdims=True)

    common = {
        "w_in_hl": w_in_hl,
        "w_gcn_hi": w_gcn_hi,
        "w_gcn_lo": w_gcn_lo,
        "b_in_row": np.ascontiguousarray(b_in.reshape(1, H)),
        "b_g3_row": np.ascontiguousarray(
            (b_gcn[L - 1] * scales[L - 1]).reshape(1, H)),
        "ones_row": np.ones((1, 128), dtype=np.float32),
    }

    in_maps = []
    for k in range(NCORES):
        sl = slots[k * NG:(k + 1) * NG]
        a_t = np.empty((NG, 128, 4, 2 * N), dtype=f8np)
        for g in range(NG):
            # [m, i] -> [q, ko, p, i] -> [p, q, (ko i)]
            at = cdfg_as[sl[g]].T.reshape(4, 2, 128, N)
            a_t[g] = at.transpose(2, 0, 1, 3).reshape(
                128, 4, 2 * N).astype(f8np)
        xs_t_hi = _rnd11(np.ascontiguousarray(cdfg_xs[sl].transpose(2, 0, 1)))
        m_t = np.zeros((128, NG * 8, B), dtype=np.float32)
        for g in range(NG):
            if real[k * NG + g]:
                rows = np.nonzero(graph == sl[g])[0]
                for b in rows:
                    m_t[:, g * 8:(g + 1) * 8, b] = mscale[b].reshape(8, 128).T
        in_maps.append({"a_t": a_t, "xs_t_hi": xs_t_hi,
                        "m_t": m_t, **common})
    return in_maps, slots, real, with_bias, inv_scales


def _assemble_out(results, graph, slots, real):
    graph = np.asarray(graph).astype(np.int64)
    out = np.zeros((B, H), dtype=np.float32)
    for k in range(NCORES):
        for g in range(NG):
            if real[k * NG + g]:
                rows = graph == slots[k * NG + g]
                out[rows] = results[k]["out"][rows]
    return out


def kernel(cdfg_xs, cdfg_as, graph, coverpoint_mask, W_in, b_in, W_gcn, b_gcn):
    from concourse.bass_utils import run_bass_kernel_spmd

    in_maps, slots, real, with_bias, inv_scales = _prepare_in_maps(
        cdfg_xs, cdfg_as, graph, coverpoint_mask, W_in, b_in, W_gcn, b_gcn)
    nc = _get_nc(with_bias, inv_scales)
    res = run_bass_kernel_spmd(nc, in_maps, core_ids=list(range(NCORES)))
    return _assemble_out(res.results, graph, slots, real)


# revision 11
# speedup vs baseline: 1.1664x; 1.0213x over previous
"""Trainium2 Bass kernel for CdfgReader GNN message passing.

Strategy:
  - Node features depend only on which CDFG a batch item references. With 64
    batch items over 32 CDFGs, run the GNN once per UNIQUE graph (<=32),
    4 graph slots per core across 8 cores. Each core emits the [64, 256]
    rows for batch items whose graph it owns; the host gathers rows.
  - Per slot: X0 = relu(xs @ W_in + b), 4 GCN layers (A @ (X @ W) + b,
    relu/tanh), residual, masked mean via a small mask matmul whose mask
    columns are pre-scaled by 1/count on the host.
  - Precision (rel tol 2e-2, ~5e-3 predicted): X@W matmuls run in f32r as
    X_hi @ (W_hi + W_lo) with W split at 11 mantissa bits. The dominant
    A-multiplies run in fp8e4 with perf_mode=DoubleRow (256-wide
    contraction at double rate): A is 0/1 (exact in fp8), and XW is split
    into two e4m3 parts (~8 effective mantissa bits) after scaling by a
    per-layer power of two (folded into W on the host, un-scaled in the
    activation) so values sit in e4m3's dynamic range.
  - Engine balance: PE matmuls; ACT does the fp8 hi-conversions + scaled
    relu/tanh; DVE does the fp8 lo-parts + input-layer relus; the
    otherwise-idle Pool engine does the residual adds.
  - Emission is software-pipelined across graphs so the PE never waits for
    activation/convert tails at layer or graph boundaries.
"""

import numpy as np

NG = 4          # graph slots per core
NCORES = 8
N = 1024        # max nodes
F = 128         # input feature dim
H = 256         # hidden dim
L = 4           # GCN layers
B = 64          # batch (coverpoints)

_CACHE = {}


def _build_nc(with_bias, inv_scales):
    import concourse.bass as bass  # noqa: F401
    import concourse.mybir as mybir
    import concourse.tile as tile
    from concourse import bacc
    from concourse.bass import ts

    f32 = mybir.dt.float32
    f32r = mybir.dt.float32r
    f8 = mybir.dt.float8e4
    DR = mybir.MatmulPerfMode.DoubleRow
    Relu = mybir.ActivationFunctionType.Relu
    Tanh = mybir.ActivationFunctionType.Tanh
    Copy = mybir.ActivationFunctionType.Copy
    sub = mybir.AluOpType.subtract

    nc = bacc.Bacc("TRN2", target_bir_lowering=False, debug=False,
                   num_devices=NCORES)

    a_t = nc.dram_tensor("a_t", [NG, 128, 4, 2 * N], f8, kind="ExternalInput")
    xs_t_hi = nc.dram_tensor("xs_t_hi", [F, NG, N], f32r, kind="ExternalInput")
    m_t = nc.dram_tensor("m_t", [128, NG * 8, B], f32r, kind="ExternalInput")
    w_in_hl = nc.dram_tensor("w_in_hl", [F, 2 * H + 2 + L * 2], f32r,
                             kind="ExternalInput")
    w_gcn_hi = nc.dram_tensor("w_gcn_hi", [128, L * 2, H], f32r,
                              kind="ExternalInput")
    w_gcn_lo = nc.dram_tensor("w_gcn_lo", [128, L * 2, H], f32r,
                              kind="ExternalInput")
    b_in_row = nc.dram_tensor("b_in_row", [1, H], f32r, kind="ExternalInput")
    b_g3_row = nc.dram_tensor("b_g3_row", [1, H], f32r, kind="ExternalInput")
    ones_row = nc.dram_tensor("ones_row", [1, 128], f32r, kind="ExternalInput")
    out = nc.dram_tensor("out", [B, H], f32, kind="ExternalOutput")

    with tile.TileContext(nc) as tc:
        with (
            tc.tile_pool(name="const", bufs=1) as constp,
            tc.tile_pool(name="apool", bufs=2) as apool,
            tc.tile_pool(name="xpool", bufs=2) as xpool,
            tc.tile_pool(name="xhpool", bufs=3) as xhpool,
            tc.tile_pool(name="xtpool", bufs=3) as xtpool,
            tc.tile_pool(name="x8pool", bufs=2) as x8pool,
            tc.tile_pool(name="psx", bufs=3, space="PSUM") as psx,
            tc.tile_pool(name="psw", bufs=4, space="PSUM") as psw,
            tc.tile_pool(name="psm", bufs=1, space="PSUM") as psm,
        ):
            # --- head DMAs in consumption order ---
            # PE warm-up during the initial DMA wait: dummy matmuls keep
            # the tensor engine "busy" so the p-state ramp completes before
            # real work arrives. Scratch is memset on the idle Pool engine.
            warm_sb = constp.tile([128, 256], f32)
            nc.gpsimd.memset(warm_sb[:], 0.0)
            for _ in range(3):
                wps = psm.tile([128, 256], mybir.dt.float32, tag="psm",
                               name="warm")
                nc.tensor.matmul(wps[:], warm_sb[:, 0:128], warm_sb[:],
                                 start=True, stop=True)

            # head: [w_in_hi (256) | w_in_lo (256) | b_pp (10)] in one DMA
            head_sb = constp.tile([128, 2 * H + 2 + L * 2], f32r)
            nc.sync.dma_start(head_sb[:], w_in_hl[:, :])
            if with_bias:
                b_in_row_sb = constp.tile([1, H], f32r)
                nc.sync.dma_start(b_in_row_sb[:], b_in_row[:, :])
                b_g3_row_sb = constp.tile([1, H], f32r)
                nc.sync.dma_start(b_g3_row_sb[:], b_g3_row[:, :])
                ones_sb = constp.tile([1, 128], f32r)
                nc.sync.dma_start(ones_sb[:], ones_row[:, :])

            w_hi_sb = constp.tile([128, L * 2, H], f32r)
            w_lo_sb = constp.tile([128, L * 2, H], f32r)
            m_t_sb = constp.tile([128, NG * 8, B], f32r)
            out_acc = constp.tile([B, H], f32)

            xs_tiles = [None] * NG
            a_tiles = [None] * NG

            def emit_dma(g):
                xs_g = xpool.tile([128, N], f32r, tag="xs", name=f"xs{g}")
                for c in range(2):
                    nc.sync.dma_start(xs_g[:, ts(c, 512)],
                                      xs_t_hi[:, g, ts(c, 512)])
                xs_tiles[g] = xs_g
                a8 = apool.tile([128, 4, 2, N], f8, tag="a", name=f"a{g}")
                if g == 0:
                    for lyr in range(2):
                        nc.sync.dma_start(w_hi_sb[:, 2 * lyr:2 * lyr + 2, :],
                                          w_gcn_hi[:, 2 * lyr:2 * lyr + 2, :])
                        nc.sync.dma_start(w_lo_sb[:, 2 * lyr:2 * lyr + 2, :],
                                          w_gcn_lo[:, 2 * lyr:2 * lyr + 2, :])
                    for q in range(4):
                        nc.sync.dma_start(
                            a8[:, q, :, :].rearrange("p ko i -> p (ko i)"),
                            a_t[0, :, q, :])
                    for lyr in range(2, L):
                        nc.sync.dma_start(w_hi_sb[:, 2 * lyr:2 * lyr + 2, :],
                                          w_gcn_hi[:, 2 * lyr:2 * lyr + 2, :])
                        nc.sync.dma_start(w_lo_sb[:, 2 * lyr:2 * lyr + 2, :],
                                          w_gcn_lo[:, 2 * lyr:2 * lyr + 2, :])
                    nc.sync.dma_start(m_t_sb[:], m_t[:, :, :])
                else:
                    nc.sync.dma_start(
                        a8[:].rearrange("p q ko i -> p q (ko i)"), a_t[g])
                a_tiles[g] = a8

            x_cur = [None] * NG
            x0n_tiles = [None] * NG
            xw_tiles = [None] * NG
            xf_tiles = [None] * NG

            def emit_p_x0t(g):
                xs_g = xs_tiles[g]
                x0t = xhpool.tile([128, 2, N], f32r, tag="xh", name=f"x0t{g}")
                for c in range(2):
                    for t in range(2):
                        ps = psx.tile([128, 512], mybir.dt.float32, tag="psx")
                        nc.tensor.matmul(ps[:], head_sb[:, ts(t, 128)],
                                         xs_g[:, ts(c, 512)],
                                         start=True, stop=False)
                        nc.tensor.matmul(ps[:],
                                         head_sb[:, 256 + t * 128:
                                                 256 + (t + 1) * 128],
                                         xs_g[:, ts(c, 512)],
                                         start=False, stop=True)
                        nc.scalar.activation(
                            x0t[:, t, ts(c, 512)], ps[:], Relu,
                            bias=head_sb[:, 512 + t:513 + t])
                x_cur[g] = x0t

            def emit_p_x0n(g):
                xs_g = xs_tiles[g]
                x0n = xpool.tile([128, 8, H], f32, tag="x0n", name=f"x0n{g}")
                for i in range(8):
                    ps = psw.tile([128, H], mybir.dt.float32, tag="psw")
                    nc.tensor.matmul(ps[:], xs_g[:, ts(i, 128)],
                     head_sb[:, 0:256],
                                     start=True, stop=not with_bias)
                    if with_bias:
                        nc.tensor.matmul(ps[:], ones_sb[:], b_in_row_sb[:],
                                         start=False, stop=True)
                    nc.scalar.activation(x0n[:, i, :], ps[:], Relu)
                x0n_tiles[g] = x0n

            def emit_w(g, layer):
                """s_l*XW = X_hi @ (W_hi + W_lo) -> split to two fp8 parts."""
                x = x_cur[g]
                xw_hi = x8pool.tile([128, 8, H], f8, tag="x8h",
                                    name=f"xw8h{g}_{layer}")
                xw_lo = x8pool.tile([128, 8, H], f8, tag="x8l",
                                    name=f"xw8l{g}_{layer}")
                for m in range(8):
                    ps = psw.tile([128, H], mybir.dt.float32, tag="psw")
                    k = 0
                    for t in range(2):
                        for w_sb in (w_hi_sb, w_lo_sb):
                            nc.tensor.matmul(
                                ps[:], x[:, t, ts(m, 128)],
                                w_sb[:, layer * 2 + t, :],
                                start=(k == 0), stop=(k == 3))
                            k += 1
                    nc.scalar.activation(xw_hi[:, m, :], ps[:], Copy)
                    nc.vector.tensor_tensor(xw_lo[:, m, :], ps[:],
                                            xw_hi[:, m, :], sub)
                xw_tiles[g] = (xw_hi, xw_lo)

            def emit_a(g, layer):
                (xw_hi, xw_lo), a8 = xw_tiles[g], a_tiles[g]
                inv_s = float(inv_scales[layer])
                if layer < L - 1:
                    # X_next^T[h, i] = sum_m XW[m, h] * A^T[m, i] (DoubleRow)
                    xn = xhpool.tile([128, 2, N], f32r, tag="xh",
                                     name=f"xn{g}_{layer}")
                    if g == 0 and layer == 0:
                        # q-outer so the matmuls chase the chunked a8 DMA;
                        # 4 psum tiles open at once (2 from psx, 2 from psw)
                        pss = [psx.tile([128, 512], mybir.dt.float32,
                                        tag="psx", name=f"a0ps{j}")
                               if j < 2 else
                               psw.tile([128, 512], mybir.dt.float32,
                                        tag="psw", name=f"a0ps{j}")
                               for j in range(4)]
                        for k, part in enumerate((xw_hi, xw_lo)):
                            for q in range(4):
                                for j, (c, t) in enumerate(
                                        ((0, 0), (0, 1), (1, 0), (1, 1))):
                                    nc.tensor.matmul(
                                        pss[j][:],
                                        part[:, 2 * q:2 * q + 2, ts(t, 128)],
                                        a8[:, q, :, ts(c, 512)],
                                        start=(k == 0 and q == 0),
                                        stop=(k == 1 and q == 3),
                                        perf_mode=DR)
                        for j, (c, t) in enumerate(
                                ((0, 0), (0, 1), (1, 0), (1, 1))):
                            nc.scalar.activation(
                                xn[:, t, ts(c, 512)], pss[j][:], Relu,
                                bias=head_sb[:, 514 + layer * 2 + t:
                                             515 + layer * 2 + t],
                                scale=inv_s)
                    else:
                        for c in range(2):
                            for t in range(2):
                                ps = psx.tile([128, 512], mybir.dt.float32,
                                              tag="psx")
                                for k, part in enumerate((xw_hi, xw_lo)):
                                    for q in range(4):
                                        nc.tensor.matmul(
                                            ps[:],
                                            part[:, 2 * q:2 * q + 2,
                                                 ts(t, 128)],
                                            a8[:, q, :, ts(c, 512)],
                                            start=(k == 0 and q == 0),
                                            stop=(k == 1 and q == 3),
                                            perf_mode=DR)
                                nc.scalar.activation(
                                    xn[:, t, ts(c, 512)], ps[:], Relu,
                                    bias=head_sb[:, 514 + layer * 2 + t:
                                                 515 + layer * 2 + t],
                                    scale=inv_s)
                    x_cur[g] = xn
                else:
                    # final layer node-major: tanh to SBUF, residual on Pool
                    xf = xpool.tile([128, 8, H], f32r, tag="xf",
                                    name=f"xf{g}")
                    x0n = x0n_tiles[g]
                    for i in range(8):
                        ps = psw.tile([128, H], mybir.dt.float32, tag="psw")
                        for k, part in enumerate((xw_hi, xw_lo)):
                            for q in range(4):
                                nc.tensor.matmul(
                                    ps[:], a8[:, q, :, ts(i, 128)],
                                    part[:, 2 * q:2 * q + 2, :],
                                    start=(k == 0 and q == 0),
                                    stop=(k == 1 and q == 3
                                          and not with_bias),
                                    perf_mode=DR)
                        if with_bias:
                            nc.tensor.matmul(ps[:], ones_sb[:],
                                             b_g3_row_sb[:],
                                             start=False, stop=True)
                        xt = xtpool.tile([128, H], f32, tag="xt3")
                        nc.scalar.activation(xt[:], ps[:], Tanh, scale=inv_s)
                        nc.vector.tensor_add(xf[:, i, :], xt[:],
                                             x0n[:, i, :])
                    xf_tiles[g] = xf

            def emit_m(g):
                """masked (pre-scaled) sums: psum[b, h] += M^T @ Xf."""
                xf = xf_tiles[g]
                pm = psm.tile([B, H], mybir.dt.float32, tag="psm")
                for c in range(8):
                    nc.tensor.matmul(pm[:], m_t_sb[:, g * 8 + c, :],
                                     xf[:, c, :], start=(c == 0), stop=(c == 7))
                if g == 0:
                    nc.vector.tensor_copy(out_acc[:], pm[:])
                else:
                    nc.vector.tensor_add(out_acc[:], out_acc[:], pm[:])

            # --- software-pipelined emission ---
            emit_dma(0)
            emit_p_x0t(0)
            emit_p_x0n(0)
            emit_w(0, 0)
            for g in range(NG):
                emit_a(g, 0)
                emit_w(g, 1)
                if g + 1 < NG:
                    emit_dma(g + 1)
                emit_a(g, 1)
                emit_w(g, 2)
                emit_a(g, 2)
                if g + 1 < NG:
                    emit_p_x0t(g + 1)
                emit_w(g, 3)
                emit_a(g, 3)
                if g + 1 < NG:
                    emit_w(g + 1, 0)
                emit_m(g)
                if g + 1 < NG:
                    emit_p_x0n(g + 1)

            nc.sync.dma_start(out[:, :], out_acc[:])

    nc.compile()
    return nc


def _get_nc(with_bias, inv_scales):
    key = ("nc", bool(with_bias), tuple(inv_scales))
    if key not in _CACHE:
        _CACHE[key] = _build_nc(with_bias, inv_scales)
    return _CACHE[key]


def _rnd11(x):
    # round-to-nearest-even at 11 explicit mantissa bits (f32r-exact)
    m, e = np.frexp(np.float32(x))
    m = np.round(m * 4096.0) / 4096.0
    return np.ldexp(m, e).astype(np.float32)


def _layer_scales(cdfg_xs, cdfg_as, uniq, W_in, b_in, W_gcn, b_gcn):
    """Power-of-two per-layer scales s_l with max|s_l * XW_l| <= ~110
    (e4m3 max is 240), from an fp32 forward pass over the unique graphs."""
    maxs = np.zeros(L, dtype=np.float64)
    for g in uniq:
        x = np.maximum(cdfg_xs[g] @ W_in + b_in, 0.0).astype(np.float32)
        a = cdfg_as[g]
        for l in range(L):
            xw = x @ W_gcn[l]
            maxs[l] = max(maxs[l], float(np.abs(xw).max()))
            h = a @ xw + b_gcn[l]
            x = (np.maximum(h, 0.0) if l < L - 1
                 else np.tanh(h)).astype(np.float32)
    s = np.exp2(np.clip(np.floor(np.log2(110.0 / np.maximum(maxs, 1e-30))),
                        -30, 30))
    return s.astype(np.float64)


def _prepare_in_maps(cdfg_xs, cdfg_as, graph, coverpoint_mask,
                     W_in, b_in, W_gcn, b_gcn):
    import concourse.mybir as mybir
    f8np = mybir.dt.np(mybir.dt.float8e4)

    cdfg_xs = np.asarray(cdfg_xs, dtype=np.float32)
    cdfg_as = np.asarray(cdfg_as, dtype=np.float32)
    graph = np.asarray(graph).astype(np.int64)
    maskf = np.asarray(coverpoint_mask).astype(np.float32)
    W_in = np.asarray(W_in, dtype=np.float32)
    b_in = np.asarray(b_in, dtype=np.float32)
    W_gcn = np.asarray(W_gcn, dtype=np.float32)
    b_gcn = np.asarray(b_gcn, dtype=np.float32)
    with_bias = bool(np.any(b_in) or np.any(b_gcn))

    uniq = np.unique(graph)
    nslots = NG * NCORES
    slots = np.empty(nslots, dtype=np.int64)
    slots[:len(uniq)] = uniq
    slots[len(uniq):] = uniq[0]
    real = np.zeros(nslots, dtype=bool)
    real[:len(uniq)] = True

    scales = _layer_scales(cdfg_xs, cdfg_as, uniq, W_in, b_in, W_gcn, b_gcn)
    inv_scales = (1.0 / scales).astype(np.float64)

    w_gcn_layout = np.ascontiguousarray(
        W_gcn.reshape(L, 2, 128, H).transpose(2, 0, 1, 3)
        .reshape(128, L * 2, H))
    w_gcn_hi = _rnd11(w_gcn_layout)
    w_gcn_lo = _rnd11(w_gcn_layout - w_gcn_hi)
    # fold the per-layer fp8 scale into the (power-of-two-exact) weights
    sc = np.repeat(scales.astype(np.float32), 2)  # [L*2]
    w_gcn_hi = np.ascontiguousarray(w_gcn_hi * sc[None, :, None])
    w_gcn_lo = np.ascontiguousarray(w_gcn_lo * sc[None, :, None])
    w_in_hi = _rnd11(W_in)
    w_in_lo = _rnd11(W_in - w_in_hi)
    b_pp = np.concatenate(
        [b_in.reshape(2, 128).T,
         b_gcn.reshape(L, 2, 128).transpose(2, 0, 1).reshape(128, L * 2)],
        axis=1)
    w_in_hl = np.ascontiguousarray(
        np.concatenate([w_in_hi, w_in_lo, b_pp], axis=1))
    mscale = maskf / maskf.sum(axis=1, keepdims=True)

    common = {
        "w_in_hl": w_in_hl,
        "w_gcn_hi": w_gcn_hi,
        "w_gcn_lo": w_gcn_lo,
        "b_in_row": np.ascontiguousarray(b_in.reshape(1, H)),
        "b_g3_row": np.ascontiguousarray(
            (b_gcn[L - 1] * scales[L - 1]).reshape(1, H)),
        "ones_row": np.ones((1, 128), dtype=np.float32),
    }

    in_maps = []
    for k in range(NCORES):
        sl = slots[k * NG:(k + 1) * NG]
        a_t = np.empty((NG, 128, 4, 2 * N), dtype=f8np)
        for g in range(NG):
            # [m, i] -> [q, ko, p, i] -> [p, q, (ko i)]
            at = cdfg_as[sl[g]].T.reshape(4, 2, 128, N)
            a_t[g] = at.transpose(2, 0, 1, 3).reshape(
                128, 4, 2 * N).astype(f8np)
        xs_t_hi = _rnd11(np.ascontiguousarray(cdfg_xs[sl].transpose(2, 0, 1)))
        m_t = np.zeros((128, NG * 8, B), dtype=np.float32)
        for g in range(NG):
            if real[k * NG + g]:
                rows = np.nonzero(graph == sl[g])[0]
                for b in rows:
                    m_t[:, g * 8:(g + 1) * 8, b] = mscale[b].reshape(8, 128).T
        in_maps.append({"a_t": a_t, "xs_t_hi": xs_t_hi,
                        "m_t": m_t, **common})
    return in_maps, slots, real, with_bias, inv_scales


def _assemble_out(results, graph, slots, real):
    graph = np.asarray(graph).astype(np.int64)
    out = np.zeros((B, H), dtype=np.float32)
    for k in range(NCORES):
        for g in range(NG):
            if real[k * NG + g]:
                rows = graph == slots[k * NG + g]
                out[rows] = results[k]["out"][rows]
    return out


def kernel(cdfg_xs, cdfg_as, graph, coverpoint_mask, W_in, b_in, W_gcn, b_gcn):
    from concourse.bass_utils import run_bass_kernel_spmd

    in_maps, slots, real, with_bias, inv_scales = _prepare_in_maps(
        cdfg_xs, cdfg_as, graph, coverpoint_mask, W_in, b_in, W_gcn, b_gcn)
    nc = _get_nc(with_bias, inv_scales)
    res = run_bass_kernel_spmd(nc, in_maps, core_ids=list(range(NCORES)))
    return _assemble_out(res.results, graph, slots, real)


# revision 13
# speedup vs baseline: 1.1958x; 1.0252x over previous
"""Trainium2 Bass kernel for CdfgReader GNN message passing.

Strategy:
  - Node features depend only on which CDFG a batch item references. With 64
    batch items over 32 CDFGs, run the GNN once per UNIQUE graph (<=32),
    4 graph slots per core across 8 cores. Each core emits the [64, 256]
    rows for batch items whose graph it owns; the host gathers rows.
  - Per slot: X0 = relu(xs @ W_in + b), 4 GCN layers (A @ (X @ W) + b,
    relu/tanh), residual, masked mean via a small mask matmul whose mask
    columns are pre-scaled by 1/count on the host.
  - Precision (rel tol 2e-2, ~5e-3 predicted): X@W matmuls run in f32r as
    X_hi @ (W_hi + W_lo) with W split at 11 mantissa bits. The dominant
    A-multiplies run in fp8e4 with perf_mode=DoubleRow (256-wide
    contraction at double rate): A is 0/1 (exact in fp8), and XW is split
    into two e4m3 parts (~8 effective mantissa bits) after scaling by a
    per-layer power of two (folded into W on the host, un-scaled in the
    activation) so values sit in e4m3's dynamic range.
  - Engine balance: PE matmuls; ACT does the fp8 hi-conversions + scaled
    relu/tanh; DVE does the fp8 lo-parts + input-layer relus; the
    otherwise-idle Pool engine does the residual adds.
  - Emission is software-pipelined across graphs so the PE never waits for
    activation/convert tails at layer or graph boundaries.
"""

import numpy as np

NG = 4          # graph slots per core
NCORES = 8
N = 1024        # max nodes
F = 128         # input feature dim
H = 256         # hidden dim
L = 4           # GCN layers
B = 64          # batch (coverpoints)

_CACHE = {}


def _build_nc(with_bias, inv_scales):
    import concourse.bass as bass  # noqa: F401
    import concourse.mybir as mybir
    import concourse.tile as tile
    from concourse import bacc
    from concourse.bass import ts

    f32 = mybir.dt.float32
    f32r = mybir.dt.float32r
    f8 = mybir.dt.float8e4
    DR = mybir.MatmulPerfMode.DoubleRow
    Relu = mybir.ActivationFunctionType.Relu
    Tanh = mybir.ActivationFunctionType.Tanh
    Copy = mybir.ActivationFunctionType.Copy
    sub = mybir.AluOpType.subtract

    nc = bacc.Bacc("TRN2", target_bir_lowering=False, debug=False,
                   num_devices=NCORES)

    a_t = nc.dram_tensor("a_t", [NG, 128, 4, 2 * N], f8, kind="ExternalInput")
    xs_t_hi = nc.dram_tensor("xs_t_hi", [F, NG, N], f32r, kind="ExternalInput")
    m_t = nc.dram_tensor("m_t", [128, NG * 8, B], f32r, kind="ExternalInput")
    w_in_hl = nc.dram_tensor("w_in_hl", [F, 2 * H + 2 + L * 2], f32r,
                             kind="ExternalInput")
    w_gcn_hi = nc.dram_tensor("w_gcn_hi", [128, L * 2, H], f32r,
                              kind="ExternalInput")
    w_gcn_lo = nc.dram_tensor("w_gcn_lo", [128, L * 2, H], f32r,
                              kind="ExternalInput")
    b_in_row = nc.dram_tensor("b_in_row", [1, H], f32r, kind="ExternalInput")
    b_g3_row = nc.dram_tensor("b_g3_row", [1, H], f32r, kind="ExternalInput")
    ones_row = nc.dram_tensor("ones_row", [1, 128], f32r, kind="ExternalInput")
    out = nc.dram_tensor("out", [B, H], f32, kind="ExternalOutput")

    with tile.TileContext(nc) as tc:
        with (
            tc.tile_pool(name="const", bufs=1) as constp,
            tc.tile_pool(name="apool", bufs=2) as apool,
            tc.tile_pool(name="xpool", bufs=2) as xpool,
            tc.tile_pool(name="xhpool", bufs=3) as xhpool,
            tc.tile_pool(name="xtpool", bufs=3) as xtpool,
            tc.tile_pool(name="x8pool", bufs=2) as x8pool,
            tc.tile_pool(name="psx", bufs=3, space="PSUM") as psx,
            tc.tile_pool(name="psw", bufs=4, space="PSUM") as psw,
            tc.tile_pool(name="psm", bufs=1, space="PSUM") as psm,
        ):
            # --- head DMAs in consumption order ---
            # PE warm-up during the initial DMA wait: dummy matmuls keep
            # the tensor engine "busy" so the p-state ramp completes before
            # real work arrives. Scratch is memset on the idle Pool engine.
            warm_sb = constp.tile([128, 256], f32)
            nc.gpsimd.memset(warm_sb[:], 0.0)
            for j in range(4):
                wps = psm.tile([128, 256], mybir.dt.float32, tag="psm",
                               name="warm")
                nc.tensor.matmul(wps[:, 0:128 if j == 3 else 256],
                                 warm_sb[:, 0:128],
                                 warm_sb[:, 0:128 if j == 3 else 256],
                                 start=True, stop=True)

            # head: [w_in_hi (256) | w_in_lo (256) | b_pp (10)] in one DMA
            head_sb = constp.tile([128, 2 * H + 2 + L * 2], f32r)
            nc.sync.dma_start(head_sb[:], w_in_hl[:, :])
            if with_bias:
                b_in_row_sb = constp.tile([1, H], f32r)
                nc.sync.dma_start(b_in_row_sb[:], b_in_row[:, :])
                b_g3_row_sb = constp.tile([1, H], f32r)
                nc.sync.dma_start(b_g3_row_sb[:], b_g3_row[:, :])
                ones_sb = constp.tile([1, 128], f32r)
                nc.sync.dma_start(ones_sb[:], ones_row[:, :])

            w_hi_sb = constp.tile([128, L * 2, H], f32r)
            w_lo_sb = constp.tile([128, L * 2, H], f32r)
            m_t_sb = constp.tile([128, NG * 8, B], f32r)
            out_acc = constp.tile([B, H], f32)

            xs_tiles = [None] * NG
            a_tiles = [None] * NG

            def emit_dma(g):
                xs_g = xpool.tile([128, N], f32r, tag="xs", name=f"xs{g}")
                if g > 0:
                    for c in range(2):
                        nc.sync.dma_start(xs_g[:, ts(c, 512)],
                                          xs_t_hi[:, g, ts(c, 512)])
                xs_tiles[g] = xs_g
                a8 = apool.tile([128, 4, 2, N], f8, tag="a", name=f"a{g}")
                if g == 0:
                    nc.sync.dma_start(xs_g[:, ts(0, 512)],
                                      xs_t_hi[:, g, ts(0, 512)])
                    nc.sync.dma_start(w_hi_sb[:, 0:2, :], w_gcn_hi[:, 0:2, :])
                    nc.sync.dma_start(w_lo_sb[:, 0:2, :], w_gcn_lo[:, 0:2, :])
                    nc.sync.dma_start(xs_g[:, ts(1, 512)],
                                      xs_t_hi[:, g, ts(1, 512)])
                    for lyr in range(1, 2):
                        nc.sync.dma_start(w_hi_sb[:, 2 * lyr:2 * lyr + 2, :],
                                          w_gcn_hi[:, 2 * lyr:2 * lyr + 2, :])
                        nc.sync.dma_start(w_lo_sb[:, 2 * lyr:2 * lyr + 2, :],
                                          w_gcn_lo[:, 2 * lyr:2 * lyr + 2, :])
                    for q in range(4):
                        nc.sync.dma_start(
                            a8[:, q, :, :].rearrange("p ko i -> p (ko i)"),
                            a_t[0, :, q, :])
                    for lyr in range(2, L):
                        nc.sync.dma_start(w_hi_sb[:, 2 * lyr:2 * lyr + 2, :],
                                          w_gcn_hi[:, 2 * lyr:2 * lyr + 2, :])
                        nc.sync.dma_start(w_lo_sb[:, 2 * lyr:2 * lyr + 2, :],
                                          w_gcn_lo[:, 2 * lyr:2 * lyr + 2, :])
                    nc.sync.dma_start(m_t_sb[:], m_t[:, :, :])
                else:
                    nc.sync.dma_start(
                        a8[:].rearrange("p q ko i -> p q (ko i)"), a_t[g])
                a_tiles[g] = a8

            x_cur = [None] * NG
            x0t_tiles = [None] * NG
            x0n_tiles = [None] * NG
            xw_tiles = [None] * NG
            xf_tiles = [None] * NG

            def emit_p_x0t(g, cs=(0, 1)):
                xs_g = xs_tiles[g]
                if x0t_tiles[g] is None:
                    x0t_tiles[g] = xhpool.tile([128, 2, N], f32r, tag="xh",
                                               name=f"x0t{g}")
                    x_cur[g] = x0t_tiles[g]
                x0t = x0t_tiles[g]
                for c in cs:
                    for t in range(2):
                        ps = psx.tile([128, 512], mybir.dt.float32, tag="psx")
                        nc.tensor.matmul(ps[:], head_sb[:, ts(t, 128)],
                                         xs_g[:, ts(c, 512)],
                                         start=True, stop=False)
                        nc.tensor.matmul(ps[:],
                                         head_sb[:, 256 + t * 128:
                                                 256 + (t + 1) * 128],
                                         xs_g[:, ts(c, 512)],
                                         start=False, stop=True)
                        nc.scalar.activation(
                            x0t[:, t, ts(c, 512)], ps[:], Relu,
                            bias=head_sb[:, 512 + t:513 + t])
            def emit_p_x0n(g, irange=range(8)):
                xs_g = xs_tiles[g]
                if x0n_tiles[g] is None:
                    x0n_tiles[g] = xpool.tile([128, 8, H], f32, tag="x0n",
                                              name=f"x0n{g}")
                x0n = x0n_tiles[g]
                for i in irange:
                    ps = psw.tile([128, H], mybir.dt.float32, tag="psw")
                    nc.tensor.matmul(ps[:], xs_g[:, ts(i, 128)],
                     head_sb[:, 0:256],
                                     start=True, stop=not with_bias)
                    if with_bias:
                        nc.tensor.matmul(ps[:], ones_sb[:], b_in_row_sb[:],
                                         start=False, stop=True)
                    nc.scalar.activation(x0n[:, i, :], ps[:], Relu)

            def emit_w(g, layer, ms=range(8)):
                """s_l*XW = X_hi @ (W_hi + W_lo) -> split to two fp8 parts."""
                x = x_cur[g]
                if ms.start == 0:
                    xw_tiles[g] = (
                        x8pool.tile([128, 8, H], f8, tag="x8h",
                                    name=f"xw8h{g}_{layer}"),
                        x8pool.tile([128, 8, H], f8, tag="x8l",
                                    name=f"xw8l{g}_{layer}"))
                xw_hi, xw_lo = xw_tiles[g]
                for m in ms:
                    ps = psw.tile([128, H], mybir.dt.float32, tag="psw")
                    k = 0
                    for t in range(2):
                        for w_sb in (w_hi_sb, w_lo_sb):
                            nc.tensor.matmul(
                                ps[:], x[:, t, ts(m, 128)],
                                w_sb[:, layer * 2 + t, :],
                                start=(k == 0), stop=(k == 3))
                            k += 1
                    nc.scalar.activation(xw_hi[:, m, :], ps[:], Copy)
                    nc.vector.tensor_tensor(xw_lo[:, m, :], ps[:],
                                            xw_hi[:, m, :], sub)

            def emit_a(g, layer):
                (xw_hi, xw_lo), a8 = xw_tiles[g], a_tiles[g]
                inv_s = float(inv_scales[layer])
                if layer < L - 1:
                    # X_next^T[h, i] = sum_m XW[m, h] * A^T[m, i] (DoubleRow)
                    xn = xhpool.tile([128, 2, N], f32r, tag="xh",
                                     name=f"xn{g}_{layer}")
                    if g == 0 and layer == 0:
                        # q-outer so the matmuls chase the chunked a8 DMA;
                        # 4 psum tiles open at once (2 from psx, 2 from psw)
                        pss = [psx.tile([128, 512], mybir.dt.float32,
                                        tag="psx", name=f"a0ps{j}")
                               if j < 2 else
                               psw.tile([128, 512], mybir.dt.float32,
                                        tag="psw", name=f"a0ps{j}")
                               for j in range(4)]
                        for k, part in enumerate((xw_hi, xw_lo)):
                            for q in range(4):
                                for j, (c, t) in enumerate(
                                        ((0, 0), (0, 1), (1, 0), (1, 1))):
                                    nc.tensor.matmul(
                                        pss[j][:],
                                        part[:, 2 * q:2 * q + 2, ts(t, 128)],
                                        a8[:, q, :, ts(c, 512)],
                                        start=(k == 0 and q == 0),
                                        stop=(k == 1 and q == 3),
                                        perf_mode=DR)
                        for j, (c, t) in enumerate(
                                ((0, 0), (0, 1), (1, 0), (1, 1))):
                            nc.scalar.activation(
                                xn[:, t, ts(c, 512)], pss[j][:], Relu,
                                bias=head_sb[:, 514 + layer * 2 + t:
                                             515 + layer * 2 + t],
                                scale=inv_s)
                    else:
                        for c in range(2):
                            for t in range(2):
                                ps = psx.tile([128, 512], mybir.dt.float32,
                                              tag="psx")
                                for k, part in enumerate((xw_hi, xw_lo)):
                                    for q in range(4):
                                        nc.tensor.matmul(
                                            ps[:],
                                            part[:, 2 * q:2 * q + 2,
                                                 ts(t, 128)],
                                            a8[:, q, :, ts(c, 512)],
                                            start=(k == 0 and q == 0),
                                            stop=(k == 1 and q == 3),
                                            perf_mode=DR)
                                if with_bias or (c * 2 + t) % 2 == 0:
                                    nc.scalar.activation(
                                        xn[:, t, ts(c, 512)], ps[:], Relu,
                                        bias=head_sb[:, 514 + layer * 2 + t:
                                                     515 + layer * 2 + t],
                                        scale=inv_s)
                                else:
                                    nc.vector.tensor_scalar(
                                        xn[:, t, ts(c, 512)], ps[:], inv_s,
                                        0.0, mybir.AluOpType.mult,
                                        mybir.AluOpType.max)
                    x_cur[g] = xn
                else:
                    # final layer node-major: tanh to SBUF, residual on Pool
                    xf = xpool.tile([128, 8, H], f32r, tag="xf",
                                    name=f"xf{g}")
                    x0n = x0n_tiles[g]
                    for i in range(8):
                        ps = psw.tile([128, H], mybir.dt.float32, tag="psw")
                        for k, part in enumerate((xw_hi, xw_lo)):
                            for q in range(4):
                                nc.tensor.matmul(
                                    ps[:], a8[:, q, :, ts(i, 128)],
                                    part[:, 2 * q:2 * q + 2, :],
                                    start=(k == 0 and q == 0),
                                    stop=(k == 1 and q == 3
                                          and not with_bias),
                                    perf_mode=DR)
                        if with_bias:
                            nc.tensor.matmul(ps[:], ones_sb[:],
                                             b_g3_row_sb[:],
                                             start=False, stop=True)
                        xt = xtpool.tile([128, H], f32, tag="xt3")
                        nc.scalar.activation(xt[:], ps[:], Tanh, scale=inv_s)
                        nc.vector.tensor_add(xf[:, i, :], xt[:],
                                             x0n[:, i, :])
                    xf_tiles[g] = xf

            def emit_m(g):
                """masked (pre-scaled) sums: psum[b, h] += M^T @ Xf."""
                xf = xf_tiles[g]
                pm = psm.tile([B, H], mybir.dt.float32, tag="psm")
                for c in range(8):
                    nc.tensor.matmul(pm[:], m_t_sb[:, g * 8 + c, :],
                                     xf[:, c, :], start=(c == 0), stop=(c == 7))
                if g == 0:
                    nc.vector.tensor_copy(out_acc[:], pm[:])
                else:
                    nc.vector.tensor_add(out_acc[:], out_acc[:], pm[:])

            # --- software-pipelined emission ---
            emit_dma(0)
            emit_p_x0t(0, cs=(0,))
            emit_p_x0n(0, irange=range(0, 4))
            emit_w(0, 0, ms=range(0, 4))
            emit_p_x0t(0, cs=(1,))
            emit_p_x0n(0, irange=range(4, 8))
            emit_w(0, 0, ms=range(4, 8))
            for g in range(NG):
                emit_a(g, 0)
                emit_w(g, 1)
                if g + 1 < NG:
                    emit_dma(g + 1)
                emit_a(g, 1)
                emit_w(g, 2)
                emit_a(g, 2)
                if g + 1 < NG:
                    emit_p_x0t(g + 1)
                emit_w(g, 3)
                emit_a(g, 3)
                if g + 1 < NG:
                    emit_w(g + 1, 0)
                emit_m(g)
                if g + 1 < NG:
                    emit_p_x0n(g + 1)

            nc.sync.dma_start(out[:, :], out_acc[:])

    nc.compile()
    return nc


def _get_nc(with_bias, inv_scales):
    key = ("nc", bool(with_bias), tuple(inv_scales))
    if key not in _CACHE:
        _CACHE[key] = _build_nc(with_bias, inv_scales)
    return _CACHE[key]


def _rnd11(x):
    # round-to-nearest-even at 11 explicit mantissa bits (f32r-exact)
    m, e = np.frexp(np.float32(x))
    m = np.round(m * 4096.0) / 4096.0
    return np.ldexp(m, e).astype(np.float32)


def _layer_scales(cdfg_xs, cdfg_as, uniq, W_in, b_in, W_gcn, b_gcn):
    """Power-of-two per-layer scales s_l with max|s_l * XW_l| <= ~110
    (e4m3 max is 240), from an fp32 forward pass over the unique graphs."""
    maxs = np.zeros(L, dtype=np.float64)
    for g in uniq:
        x = np.maximum(cdfg_xs[g] @ W_in + b_in, 0.0).astype(np.float32)
        a = cdfg_as[g]
        for l in range(L):
            xw = x @ W_gcn[l]
            maxs[l] = max(maxs[l], float(np.abs(xw).max()))
            h = a @ xw + b_gcn[l]
            x = (np.maximum(h, 0.0) if l < L - 1
                 else np.tanh(h)).astype(np.float32)
    s = np.exp2(np.clip(np.floor(np.log2(110.0 / np.maximum(maxs, 1e-30))),
                        -30, 30))
    return s.astype(np.float64)


def _prepare_in_maps(cdfg_xs, cdfg_as, graph, coverpoint_mask,
                     W_in, b_in, W_gcn, b_gcn):
    import concourse.mybir as mybir
    f8np = mybir.dt.np(mybir.dt.float8e4)

    cdfg_xs = np.asarray(cdfg_xs, dtype=np.float32)
    cdfg_as = np.asarray(cdfg_as, dtype=np.float32)
    graph = np.asarray(graph).astype(np.int64)
    maskf = np.asarray(coverpoint_mask).astype(np.float32)
    W_in = np.asarray(W_in, dtype=np.float32)
    b_in = np.asarray(b_in, dtype=np.float32)
    W_gcn = np.asarray(W_gcn, dtype=np.float32)
    b_gcn = np.asarray(b_gcn, dtype=np.float32)
    with_bias = bool(np.any(b_in) or np.any(b_gcn))

    uniq = np.unique(graph)
    nslots = NG * NCORES
    slots = np.empty(nslots, dtype=np.int64)
    slots[:len(uniq)] = uniq
    slots[len(uniq):] = uniq[0]
    real = np.zeros(nslots, dtype=bool)
    real[:len(uniq)] = True

    scales = _layer_scales(cdfg_xs, cdfg_as, uniq, W_in, b_in, W_gcn, b_gcn)
    inv_scales = (1.0 / scales).astype(np.float64)

    w_gcn_layout = np.ascontiguousarray(
        W_gcn.reshape(L, 2, 128, H).transpose(2, 0, 1, 3)
        .reshape(128, L * 2, H))
    w_gcn_hi = _rnd11(w_gcn_layout)
    w_gcn_lo = _rnd11(w_gcn_layout - w_gcn_hi)
    # fold the per-layer fp8 scale into the (power-of-two-exact) weights
    sc = np.repeat(scales.astype(np.float32), 2)  # [L*2]
    w_gcn_hi = np.ascontiguousarray(w_gcn_hi * sc[None, :, None])
    w_gcn_lo = np.ascontiguousarray(w_gcn_lo * sc[None, :, None])
    w_in_hi = _rnd11(W_in)
    w_in_lo = _rnd11(W_in - w_in_hi)
    b_pp = np.concatenate(
        [b_in.reshape(2, 128).T,
         b_gcn.reshape(L, 2, 128).transpose(2, 0, 1).reshape(128, L * 2)],
        axis=1)
    w_in_hl = np.ascontiguousarray(
        np.concatenate([w_in_hi, w_in_lo, b_pp], axis=1))
    mscale = maskf / maskf.sum(axis=1, keepdims=True)

    common = {
        "w_in_hl": w_in_hl,
        "w_gcn_hi": w_gcn_hi,
        "w_gcn_lo": w_gcn_lo,
        "b_in_row": np.ascontiguousarray(b_in.reshape(1, H)),
        "b_g3_row": np.ascontiguousarray(
            (b_gcn[L - 1] * scales[L - 1]).reshape(1, H)),
        "ones_row": np.ones((1, 128), dtype=np.float32),
    }

    in_maps = []
    for k in range(NCORES):
        sl = slots[k * NG:(k + 1) * NG]
        a_t = np.empty((NG, 128, 4, 2 * N), dtype=f8np)
        for g in range(NG):
            # [m, i] -> [q, ko, p, i] -> [p, q, (ko i)]
            at = cdfg_as[sl[g]].T.reshape(4, 2, 128, N)
            a_t[g] = at.transpose(2, 0, 1, 3).reshape(
                128, 4, 2 * N).astype(f8np)
        xs_t_hi = _rnd11(np.ascontiguousarray(cdfg_xs[sl].transpose(2, 0, 1)))
        m_t = np.zeros((128, NG * 8, B), dtype=np.float32)
        for g in range(NG):
            if real[k * NG + g]:
                rows = np.nonzero(graph == sl[g])[0]
                for b in rows:
                    m_t[:, g * 8:(g + 1) * 8, b] = mscale[b].reshape(8, 128).T
        in_maps.append({"a_t": a_t, "xs_t_hi": xs_t_hi,
                        "m_t": m_t, **common})
    return in_maps, slots, real, with_bias, inv_scales


def _assemble_out(results, graph, slots, real):
    graph = np.asarray(graph).astype(np.int64)
    out = np.zeros((B, H), dtype=np.float32)
    for k in range(NCORES):
        for g in range(NG):
            if real[k * NG + g]:
                rows = graph == slots[k * NG + g]
                out[rows] = results[k]["out"][rows]
    return out


def kernel(cdfg_xs, cdfg_as, graph, coverpoint_mask, W_in, b_in, W_gcn, b_gcn):
    from concourse.bass_utils import run_bass_kernel_spmd

    in_maps, slots, real, with_bias, inv_scales = _prepare_in_maps(
        cdfg_xs, cdfg_as, graph, coverpoint_mask, W_in, b_in, W_gcn, b_gcn)
    nc = _get_nc(with_bias, inv_scales)
    res = run_bass_kernel_spmd(nc, in_maps, core_ids=list(range(NCORES)))
    return _assemble_out(res.results, graph, slots, real)


# revision 14
# speedup vs baseline: 1.2107x; 1.0125x over previous
"""Trainium2 Bass kernel for CdfgReader GNN message passing.

Strategy:
  - Node features depend only on which CDFG a batch item references. With 64
    batch items over 32 CDFGs, run the GNN once per UNIQUE graph (<=32),
    4 graph slots per core across 8 cores. Each core emits the [64, 256]
    rows for batch items whose graph it owns; the host gathers rows.
  - Per slot: X0 = relu(xs @ W_in + b), 4 GCN layers (A @ (X @ W) + b,
    relu/tanh), residual, masked mean via a small mask matmul whose mask
    columns are pre-scaled by 1/count on the host.
  - Precision (rel tol 2e-2, ~5e-3 predicted): X@W matmuls run in f32r as
    X_hi @ (W_hi + W_lo) with W split at 11 mantissa bits. The dominant
    A-multiplies run in fp8e4 with perf_mode=DoubleRow (256-wide
    contraction at double rate): A is 0/1 (exact in fp8), and XW is split
    into two e4m3 parts (~8 effective mantissa bits) after scaling by a
    per-layer power of two (folded into W on the host, un-scaled in the
    activation) so values sit in e4m3's dynamic range.
  - Engine balance: PE matmuls; ACT does the fp8 hi-conversions + scaled
    relu/tanh; DVE does the fp8 lo-parts + input-layer relus; the
    otherwise-idle Pool engine does the residual adds.
  - Emission is software-pipelined across graphs so the PE never waits for
    activation/convert tails at layer or graph boundaries.
"""

import numpy as np

NG = 4          # graph slots per core
NCORES = 8
N = 1024        # max nodes
F = 128         # input feature dim
H = 256         # hidden dim
L = 4           # GCN layers
B = 64          # batch (coverpoints)

_CACHE = {}


def _build_nc(with_bias, inv_scales):
    import concourse.bass as bass  # noqa: F401
    import concourse.mybir as mybir
    import concourse.tile as tile
    from concourse import bacc
    from concourse.bass import ts

    f32 = mybir.dt.float32
    f32r = mybir.dt.float32r
    f8 = mybir.dt.float8e4
    DR = mybir.MatmulPerfMode.DoubleRow
    Relu = mybir.ActivationFunctionType.Relu
    Tanh = mybir.ActivationFunctionType.Tanh
    Copy = mybir.ActivationFunctionType.Copy
    sub = mybir.AluOpType.subtract

    nc = bacc.Bacc("TRN2", target_bir_lowering=False, debug=False,
                   num_devices=NCORES)

    a_t = nc.dram_tensor("a_t", [NG, 128, 4, 2 * N], f8, kind="ExternalInput")
    xs_t_hi = nc.dram_tensor("xs_t_hi", [F, NG, N], f32r, kind="ExternalInput")
    m_t = nc.dram_tensor("m_t", [128, NG * 8, B], f32r, kind="ExternalInput")
    w_in_hl = nc.dram_tensor("w_in_hl", [F, H + 2 + L * 2], f32r,
                             kind="ExternalInput")
    w_gcn = nc.dram_tensor("w_gcn", [128, L, 4 * H], f32r,
                           kind="ExternalInput")
    b_in_row = nc.dram_tensor("b_in_row", [1, H], f32r, kind="ExternalInput")
    b_g3_row = nc.dram_tensor("b_g3_row", [1, H], f32r, kind="ExternalInput")
    ones_row = nc.dram_tensor("ones_row", [1, 128], f32r, kind="ExternalInput")
    out = nc.dram_tensor("out", [B, H], f32, kind="ExternalOutput")

    with tile.TileContext(nc) as tc:
        with (
            tc.tile_pool(name="const", bufs=1) as constp,
            tc.tile_pool(name="apool", bufs=2) as apool,
            tc.tile_pool(name="xpool", bufs=2) as xpool,
            tc.tile_pool(name="xhpool", bufs=3) as xhpool,
            tc.tile_pool(name="xtpool", bufs=3) as xtpool,
            tc.tile_pool(name="x8pool", bufs=2) as x8pool,
            tc.tile_pool(name="psx", bufs=3, space="PSUM") as psx,
            tc.tile_pool(name="psw", bufs=4, space="PSUM") as psw,
            tc.tile_pool(name="psm", bufs=1, space="PSUM") as psm,
        ):
            # --- head DMAs in consumption order ---
            # PE warm-up during the initial DMA wait: dummy matmuls keep
            # the tensor engine "busy" so the p-state ramp completes before
            # real work arrives. Scratch is memset on the idle Pool engine.
            warm_sb = constp.tile([128, 256], f32)
            nc.vector.memset(warm_sb[:], 0.0)
            for j in range(4):
                wps = psm.tile([128, 256], mybir.dt.float32, tag="psm",
                               name="warm")
                nc.tensor.matmul(wps[:, 0:128 if j == 3 else 256],
                                 warm_sb[:, 0:128],
                                 warm_sb[:, 0:128 if j == 3 else 256],
                                 start=True, stop=True)

            # head: [w_in_hi (256) | b_pp (10)] in one DMA
            head_sb = constp.tile([128, H + 2 + L * 2], f32r)
            nc.sync.dma_start(head_sb[:], w_in_hl[:, :])
            if with_bias:
                b_in_row_sb = constp.tile([1, H], f32r)
                nc.sync.dma_start(b_in_row_sb[:], b_in_row[:, :])
                b_g3_row_sb = constp.tile([1, H], f32r)
                nc.sync.dma_start(b_g3_row_sb[:], b_g3_row[:, :])
                ones_sb = constp.tile([1, 128], f32r)
                nc.sync.dma_start(ones_sb[:], ones_row[:, :])

            wg_sb = constp.tile([128, L, 4 * H], f32r)
            m_t_sb = constp.tile([128, NG * 8, B], f32r)
            out_acc = constp.tile([B, H], f32)

            xs_tiles = [None] * NG
            a_tiles = [None] * NG

            def emit_dma(g):
                xs_g = xpool.tile([128, N], f32r, tag="xs", name=f"xs{g}")
                if g > 0:
                    for c in range(2):
                        nc.sync.dma_start(xs_g[:, ts(c, 512)],
                                          xs_t_hi[:, g, ts(c, 512)])
                xs_tiles[g] = xs_g
                a8 = apool.tile([128, 4, 2, N], f8, tag="a", name=f"a{g}")
                if g == 0:
                    nc.sync.dma_start(xs_g[:, ts(0, 512)],
                                      xs_t_hi[:, g, ts(0, 512)])
                    nc.sync.dma_start(wg_sb[:, 0, :], w_gcn[:, 0, :])
                    nc.sync.dma_start(xs_g[:, ts(1, 512)],
                                      xs_t_hi[:, g, ts(1, 512)])
                    nc.sync.dma_start(wg_sb[:, 1, :], w_gcn[:, 1, :])
                    for q in range(4):
                        nc.sync.dma_start(
                            a8[:, q, :, :].rearrange("p ko i -> p (ko i)"),
                            a_t[0, :, q, :])
                    for lyr in range(2, L):
                        nc.sync.dma_start(wg_sb[:, lyr, :], w_gcn[:, lyr, :])
                    nc.sync.dma_start(m_t_sb[:], m_t[:, :, :])
                else:
                    nc.sync.dma_start(
                        a8[:].rearrange("p q ko i -> p q (ko i)"), a_t[g])
                a_tiles[g] = a8

            x_cur = [None] * NG
            x0t_tiles = [None] * NG
            x0n_tiles = [None] * NG
            xw_tiles = [None] * NG
            xf_tiles = [None] * NG

            def emit_p_x0t(g, cs=(0, 1)):
                xs_g = xs_tiles[g]
                if x0t_tiles[g] is None:
                    x0t_tiles[g] = xhpool.tile([128, 2, N], f32r, tag="xh",
                                               name=f"x0t{g}")
                    x_cur[g] = x0t_tiles[g]
                x0t = x0t_tiles[g]
                for c in cs:
                    for t in range(2):
                        ps = psx.tile([128, 512], mybir.dt.float32, tag="psx")
                        nc.tensor.matmul(ps[:], head_sb[:, ts(t, 128)],
                                         xs_g[:, ts(c, 512)],
                                         start=True, stop=True)
                        nc.scalar.activation(
                            x0t[:, t, ts(c, 512)], ps[:], Relu,
                            bias=head_sb[:, 256 + t:257 + t])
            def emit_p_x0n(g, irange=range(8)):
                xs_g = xs_tiles[g]
                if x0n_tiles[g] is None:
                    x0n_tiles[g] = xpool.tile([128, 8, H], f32, tag="x0n",
                                              name=f"x0n{g}")
                x0n = x0n_tiles[g]
                for i in irange:
                    ps = psw.tile([128, H], mybir.dt.float32, tag="psw")
                    nc.tensor.matmul(ps[:], xs_g[:, ts(i, 128)],
                     head_sb[:, 0:256],
                                     start=True, stop=not with_bias)
                    if with_bias:
                        nc.tensor.matmul(ps[:], ones_sb[:], b_in_row_sb[:],
                                         start=False, stop=True)
                        nc.scalar.activation(x0n[:, i, :], ps[:], Relu)
                    else:
                        nc.vector.tensor_scalar_max(x0n[:, i, :], ps[:], 0.0)

            def emit_w(g, layer, ms=range(8)):
                """s_l*XW = X_hi @ (W_hi + W_lo) -> split to two fp8 parts."""
                x = x_cur[g]
                if ms.start == 0:
                    xw_tiles[g] = (
                        x8pool.tile([128, 8, H], f8, tag="x8h",
                                    name=f"xw8h{g}_{layer}"),
                        x8pool.tile([128, 8, H], f8, tag="x8l",
                                    name=f"xw8l{g}_{layer}"))
                xw_hi, xw_lo = xw_tiles[g]
                for m in ms:
                    ps = psw.tile([128, H], mybir.dt.float32, tag="psw")
                    k = 0
                    for t in range(2):
                        for off in (t * 256, 512 + t * 256):
                            nc.tensor.matmul(
                                ps[:], x[:, t, ts(m, 128)],
                                wg_sb[:, layer, off:off + 256],
                                start=(k == 0), stop=(k == 3))
                            k += 1
                    nc.scalar.activation(xw_hi[:, m, :], ps[:], Copy)
                    nc.vector.tensor_tensor(xw_lo[:, m, :], ps[:],
                                            xw_hi[:, m, :], sub)

            def emit_a(g, layer):
                (xw_hi, xw_lo), a8 = xw_tiles[g], a_tiles[g]
                inv_s = float(inv_scales[layer])
                if layer < L - 1:
                    # X_next^T[h, i] = sum_m XW[m, h] * A^T[m, i] (DoubleRow)
                    xn = xhpool.tile([128, 2, N], f32r, tag="xh",
                                     name=f"xn{g}_{layer}")
                    if g == 0 and layer == 0:
                        # q-outer so the matmuls chase the chunked a8 DMA;
                        # 4 psum tiles open at once (2 from psx, 2 from psw)
                        pss = [psx.tile([128, 512], mybir.dt.float32,
                                        tag="psx", name=f"a0ps{j}")
                               if j < 2 else
                               psw.tile([128, 512], mybir.dt.float32,
                                        tag="psw", name=f"a0ps{j}")
                               for j in range(4)]
                        for k, part in enumerate((xw_hi, xw_lo)):
                            for q in range(4):
                                for j, (c, t) in enumerate(
                                        ((0, 0), (0, 1), (1, 0), (1, 1))):
                                    nc.tensor.matmul(
                                        pss[j][:],
                                        part[:, 2 * q:2 * q + 2, ts(t, 128)],
                                        a8[:, q, :, ts(c, 512)],
                                        start=(k == 0 and q == 0),
                                        stop=(k == 1 and q == 3),
                                        perf_mode=DR)
                        for j, (c, t) in enumerate(
                                ((0, 0), (0, 1), (1, 0), (1, 1))):
                            nc.scalar.activation(
                                xn[:, t, ts(c, 512)], pss[j][:], Relu,
                                bias=head_sb[:, 258 + layer * 2 + t:
                                             259 + layer * 2 + t],
                                scale=inv_s)
                    else:
                        for c in range(2):
                            for t in range(2):
                                ps = psx.tile([128, 512], mybir.dt.float32,
                                              tag="psx")
                                for k, part in enumerate((xw_hi, xw_lo)):
                                    for q in range(4):
                                        nc.tensor.matmul(
                                            ps[:],
                                            part[:, 2 * q:2 * q + 2,
                                                 ts(t, 128)],
                                            a8[:, q, :, ts(c, 512)],
                                            start=(k == 0 and q == 0),
                                            stop=(k == 1 and q == 3),
                                            perf_mode=DR)
                                if with_bias or (c * 2 + t) % 2 == 0:
                                    nc.scalar.activation(
                                        xn[:, t, ts(c, 512)], ps[:], Relu,
                                        bias=head_sb[:, 258 + layer * 2 + t:
                                                     259 + layer * 2 + t],
                                        scale=inv_s)
                                else:
                                    nc.vector.tensor_scalar(
                                        xn[:, t, ts(c, 512)], ps[:], inv_s,
                                        0.0, mybir.AluOpType.mult,
                                        mybir.AluOpType.max)
                    x_cur[g] = xn
                else:
                    # final layer node-major: tanh to SBUF, residual on Pool
                    xf = xpool.tile([128, 8, H], f32r, tag="xf",
                                    name=f"xf{g}")
                    x0n = x0n_tiles[g]
                    for i in range(8):
                        ps = psw.tile([128, H], mybir.dt.float32, tag="psw")
                        for k, part in enumerate((xw_hi, xw_lo)):
                            for q in range(4):
                                nc.tensor.matmul(
                                    ps[:], a8[:, q, :, ts(i, 128)],
                                    part[:, 2 * q:2 * q + 2, :],
                                    start=(k == 0 and q == 0),
                                    stop=(k == 1 and q == 3
                                          and not with_bias),
                                    perf_mode=DR)
                        if with_bias:
                            nc.tensor.matmul(ps[:], ones_sb[:],
                                             b_g3_row_sb[:],
                                             start=False, stop=True)
                        xt = xtpool.tile([128, H], f32, tag="xt3")
                        nc.scalar.activation(xt[:], ps[:], Tanh, scale=inv_s)
                        nc.vector.tensor_add(xf[:, i, :], xt[:],
                                             x0n[:, i, :])
                    xf_tiles[g] = xf

            def emit_m(g):
                """masked (pre-scaled) sums: psum[b, h] += M^T @ Xf."""
                xf = xf_tiles[g]
                pm = psm.tile([B, H], mybir.dt.float32, tag="psm")
                for c in range(8):
                    nc.tensor.matmul(pm[:], m_t_sb[:, g * 8 + c, :],
                                     xf[:, c, :], start=(c == 0), stop=(c == 7))
                if g == 0:
                    nc.vector.tensor_copy(out_acc[:], pm[:])
                else:
                    nc.vector.tensor_add(out_acc[:], out_acc[:], pm[:])

            # --- software-pipelined emission ---
            emit_dma(0)
            emit_p_x0t(0, cs=(0,))
            emit_p_x0n(0, irange=range(0, 4))
            emit_w(0, 0, ms=range(0, 4))
            emit_p_x0t(0, cs=(1,))
            emit_p_x0n(0, irange=range(4, 8))
            emit_w(0, 0, ms=range(4, 8))
            for g in range(NG):
                emit_a(g, 0)
                emit_w(g, 1)
                if g + 1 < NG:
                    emit_dma(g + 1)
                emit_a(g, 1)
                emit_w(g, 2)
                emit_a(g, 2)
                if g + 1 < NG:
                    emit_p_x0t(g + 1)
                emit_w(g, 3)
                emit_a(g, 3)
                if g + 1 < NG:
                    emit_w(g + 1, 0)
                emit_m(g)
                if g + 1 < NG:
                    emit_p_x0n(g + 1)

            nc.sync.dma_start(out[:, :], out_acc[:])

    nc.compile()
    return nc


def _get_nc(with_bias, inv_scales):
    key = ("nc", bool(with_bias), tuple(inv_scales))
    if key not in _CACHE:
        _CACHE[key] = _build_nc(with_bias, inv_scales)
    return _CACHE[key]


def _rnd11(x):
    # round-to-nearest-even at 11 explicit mantissa bits (f32r-exact)
    m, e = np.frexp(np.float32(x))
    m = np.round(m * 4096.0) / 4096.0
    return np.ldexp(m, e).astype(np.float32)


def _layer_scales(cdfg_xs, cdfg_as, uniq, W_in, b_in, W_gcn, b_gcn):
    """Power-of-two per-layer scales s_l with max|s_l * XW_l| <= ~110
    (e4m3 max is 240), from an fp32 forward pass over the unique graphs."""
    maxs = np.zeros(L, dtype=np.float64)
    for g in uniq:
        x = np.maximum(cdfg_xs[g] @ W_in + b_in, 0.0).astype(np.float32)
        a = cdfg_as[g]
        for l in range(L):
            xw = x @ W_gcn[l]
            maxs[l] = max(maxs[l], float(np.abs(xw).max()))
            h = a @ xw + b_gcn[l]
            x = (np.maximum(h, 0.0) if l < L - 1
                 else np.tanh(h)).astype(np.float32)
    s = np.exp2(np.clip(np.floor(np.log2(110.0 / np.maximum(maxs, 1e-30))),
                        -30, 30))
    return s.astype(np.float64)


def _prepare_in_maps(cdfg_xs, cdfg_as, graph, coverpoint_mask,
                     W_in, b_in, W_gcn, b_gcn):
    import concourse.mybir as mybir
    f8np = mybir.dt.np(mybir.dt.float8e4)

    cdfg_xs = np.asarray(cdfg_xs, dtype=np.float32)
    cdfg_as = np.asarray(cdfg_as, dtype=np.float32)
    graph = np.asarray(graph).astype(np.int64)
    maskf = np.asarray(coverpoint_mask).astype(np.float32)
    W_in = np.asarray(W_in, dtype=np.float32)
    b_in = np.asarray(b_in, dtype=np.float32)
    W_gcn = np.asarray(W_gcn, dtype=np.float32)
    b_gcn = np.asarray(b_gcn, dtype=np.float32)
    with_bias = bool(np.any(b_in) or np.any(b_gcn))

    uniq = np.unique(graph)
    nslots = NG * NCORES
    slots = np.empty(nslots, dtype=np.int64)
    slots[:len(uniq)] = uniq
    slots[len(uniq):] = uniq[0]
    real = np.zeros(nslots, dtype=bool)
    real[:len(uniq)] = True

    scales = _layer_scales(cdfg_xs, cdfg_as, uniq, W_in, b_in, W_gcn, b_gcn)
    inv_scales = (1.0 / scales).astype(np.float64)

    w_gcn_layout = np.ascontiguousarray(
        W_gcn.reshape(L, 2, 128, H).transpose(2, 0, 1, 3)
        .reshape(128, L * 2, H))
    w_gcn_hi = _rnd11(w_gcn_layout)
    w_gcn_lo = _rnd11(w_gcn_layout - w_gcn_hi)
    # fold the per-layer fp8 scale into the (power-of-two-exact) weights
    sc = np.repeat(scales.astype(np.float32), 2)  # [L*2]
    w_gcn_hi = w_gcn_hi * sc[None, :, None]
    w_gcn_lo = w_gcn_lo * sc[None, :, None]
    # pack per layer: [hi_t0 | hi_t1 | lo_t0 | lo_t1] each [128, 256]
    w_gcn_pk = np.ascontiguousarray(np.concatenate(
        [w_gcn_hi.reshape(128, L, 2 * H), w_gcn_lo.reshape(128, L, 2 * H)],
        axis=2))
    w_in_hi = _rnd11(W_in)
    b_pp = np.concatenate(
        [b_in.reshape(2, 128).T,
         b_gcn.reshape(L, 2, 128).transpose(2, 0, 1).reshape(128, L * 2)],
        axis=1)
    w_in_hl = np.ascontiguousarray(
        np.concatenate([w_in_hi, b_pp], axis=1))
    mscale = maskf / maskf.sum(axis=1, keepdims=True)

    common = {
        "w_in_hl": w_in_hl,
        "w_gcn": w_gcn_pk,
        "b_in_row": np.ascontiguousarray(b_in.reshape(1, H)),
        "b_g3_row": np.ascontiguousarray(
            (b_gcn[L - 1] * scales[L - 1]).reshape(1, H)),
        "ones_row": np.ones((1, 128), dtype=np.float32),
    }

    in_maps = []
    for k in range(NCORES):
        sl = slots[k * NG:(k + 1) * NG]
        a_t = np.empty((NG, 128, 4, 2 * N), dtype=f8np)
        for g in range(NG):
            # [m, i] -> [q, ko, p, i] -> [p, q, (ko i)]
            at = cdfg_as[sl[g]].T.reshape(4, 2, 128, N)
            a_t[g] = at.transpose(2, 0, 1, 3).reshape(
                128, 4, 2 * N).astype(f8np)
        xs_t_hi = _rnd11(np.ascontiguousarray(cdfg_xs[sl].transpose(2, 0, 1)))
        m_t = np.zeros((128, NG * 8, B), dtype=np.float32)
        for g in range(NG):
            if real[k * NG + g]:
                rows = np.nonzero(graph == sl[g])[0]
                for b in rows:
                    m_t[:, g * 8:(g + 1) * 8, b] = mscale[b].reshape(8, 128).T
        in_maps.append({"a_t": a_t, "xs_t_hi": xs_t_hi,
                        "m_t": m_t, **common})
    return in_maps, slots, real, with_bias, inv_scales


def _assemble_out(results, graph, slots, real):
    graph = np.asarray(graph).astype(np.int64)
    out = np.zeros((B, H), dtype=np.float32)
    for k in range(NCORES):
        for g in range(NG):
            if real[k * NG + g]:
                rows = graph == slots[k * NG + g]
                out[rows] = results[k]["out"][rows]
    return out


def kernel(cdfg_xs, cdfg_as, graph, coverpoint_mask, W_in, b_in, W_gcn, b_gcn):
    from concourse.bass_utils import run_bass_kernel_spmd

    in_maps, slots, real, with_bias, inv_scales = _prepare_in_maps(
        cdfg_xs, cdfg_as, graph, coverpoint_mask, W_in, b_in, W_gcn, b_gcn)
    nc = _get_nc(with_bias, inv_scales)
    res = run_bass_kernel_spmd(nc, in_maps, core_ids=list(range(NCORES)))
    return _assemble_out(res.results, graph, slots, real)
